# revision 1
# baseline (speedup 1.0000x reference)
"""Causal-attention (QKV projection + softmax(QK^T/sqrt(d))V) on 8 trn2 cores.

Contract: kernel(x, Wq, Wk, Wv) takes FULL inputs
  x [4, 4096, 768] f32, Wq/Wk/Wv [768, 128] f32
and returns the FULL output [4, 4096, 128] f32.

Sharding: 2 cores per batch. Core with parity h in {0,1} of batch b owns query
rows h::2 (perfect causal load balance). The host permutes the per-core input
to xT_p = concat(x[b, h::2], x[b, 1-h::2]).T so one compiled SPMD program runs
on every core; causality of the permuted key order is enforced with per-core
additive-mask data folded into the scores matmuls (identity-stationary
accumulating matmul adds -1000 on disallowed pairs; exp underflows to 0).

Per-core device program (fp16 matmuls, fp32 PSUM accumulation):
  K^T[d=128, S], Q^T[d=128, S/2], V[key-tile][128 keys, 128 d] projections;
  per 512-query tile: scores^T tiles [128 keys, 512 q] -> exp on ScalarE
  (no max subtraction: scores ~ N(0,1)) -> AV accumulated in PSUM.
  Outputs: numerator OUT^T [128, S/2] f32 and exp-sum tiles [128, 2*S/2] f16;
  the host reduces the exp-sums to denominators, divides, and scatters.
"""
import numpy as np

import concourse.bass as bass
import concourse.mybir as mybir
import concourse.tile as tile_mod
from concourse.tile import ScopedClock, VectorClock
from concourse.tile_sem_assignment import N_PROCS
from concourse.bass_utils import run_bass_kernel_spmd

f32 = mybir.dt.float32
f16 = mybir.dt.float16

B, S, D_IN, D = 4, 4096, 768, 128
N_DIN = D_IN // 128  # 6
TQ = 512             # queries per q-tile
SCALE = 1.0 / np.sqrt(np.float32(D))
AF = mybir.ActivationFunctionType

# ---------------------------------------------------------------------------
# Workarounds: the walrus build in this container accepts only ONE sync-wait
# command per instruction. TileContext's exit drain carries one wait per
# active proc, and Tile's sem assignment emits multi-wait instructions.
# Split both onto single-wait carrier instructions.
# ---------------------------------------------------------------------------


def _split_drain_and_barrier(self, tick_clock, wait_clock):
    gc = tick_clock.global_clock
    for p in range(N_PROCS):
        if gc[p] == 0:
            continue
        vc = VectorClock([gc[q] if q == p else 0 for q in range(N_PROCS)])
        d = self.nc.sync.drain()
        wait_clock.add_sem_waits(d.ins, ScopedClock({None: vc}))
    self.nc.all_engine_barrier()
    assert self.sems is not None
    popped = self.nc._tile_sem_poison_stack.pop()
    assert popped is self._sem_poison
    self.nc.clear_and_free_semaphores(list(self.sems.allocated().values()))
    self.nc.all_engine_barrier()


tile_mod.TileContext._drain_and_barrier = _split_drain_and_barrier


def _split_waits(nc, max_waits=1):
    for fn in nc.m.functions:
        for bb in fn.blocks:
            insts = bb.instructions
            if not any(
                i.sync_info and i.sync_info.on_wait
                and len(i.sync_info.on_wait) > max_waits
                for i in insts
            ):
                continue
            new = []
            for inst in insts:
                si = inst.sync_info
                ow = list(si.on_wait) if si and si.on_wait else []
                if len(ow) > max_waits:
                    excess, keep = ow[:-max_waits], ow[-max_waits:]
                    for j, w in enumerate(excess):
                        new.append(
                            mybir.InstEventSemaphore(
                                name=f"{inst.name}-wsplit{j}",
                                engine=inst.engine,
                                ins=[],
                                outs=[],
                                sync_info=mybir.SyncInfo(
                                    on_wait=[w], on_update=[]
                                ),
                            )
                        )
                    inst.sync_info = mybir.SyncInfo(
                        on_wait=keep, on_update=list(si.on_update or [])
                    )
                new.append(inst)
            bb.instructions = new


# ---------------------------------------------------------------------------
# Device program
# ---------------------------------------------------------------------------


def _build():
    NQ = S // 2
    n_qt = NQ // TQ
    n_kt_half = NQ // 128

    nc = bass.Bass()
    xT = nc.declare_dram_parameter("xT", [D_IN, S], f16, isOutput=False)
    W = nc.declare_dram_parameter("W", [128, N_DIN * 3 * D], f16, isOutput=False)
    mask = nc.declare_dram_parameter("mask", [128, 8 * TQ + 128], f16, isOutput=False)
    out_num = nc.declare_dram_parameter("out_num", [D, NQ], f32, isOutput=True)
    out_den = nc.declare_dram_parameter("out_den", [128, 2 * NQ], f16, isOutput=True)

    with tile_mod.TileContext(nc) as tc:
        with (
            tc.tile_pool(name="persist", bufs=1) as persist,
            tc.tile_pool(name="work", bufs=6) as work,
            tc.tile_pool(name="sacc_p", bufs=2) as sacc_p,
            tc.tile_pool(name="outp", bufs=2) as outp,
            tc.tile_pool(name="ps_big", bufs=2, space="PSUM") as ps_big,
            tc.tile_pool(name="ps_out", bufs=2, space="PSUM") as ps_out,
            tc.tile_pool(name="ps_sml", bufs=2, space="PSUM") as ps_sml,
        ):
            x_sb = [persist.tile([128, S], f16, tag=f"x{di}", name=f"x{di}")
                    for di in range(N_DIN)]
            w_all = persist.tile([128, N_DIN * 3 * D], f16, tag="w_all")
            m_all = persist.tile([128, 8 * TQ + 128], f16, tag="m_all")
            kt_sb = [persist.tile([128, 512], f16, tag=f"kt{c}", name=f"kt{c}")
                     for c in range(S // 512)]
            qt_sb = [persist.tile([128, TQ], f16, tag=f"qt{t}", name=f"qt{t}")
                     for t in range(n_qt)]
            v_sb = [persist.tile([128, D], f16, tag=f"v{k}", name=f"v{k}")
                    for k in range(2 * n_kt_half)]

            w_sb = [w_all[:, 3 * D * di:3 * D * (di + 1)] for di in range(N_DIN)]
            m_sb = [m_all[:, TQ * r:TQ * (r + 1)] for r in range(8)]
            ident = m_all[:, 8 * TQ:8 * TQ + 128]

            # input DMAs: W/masks first, then x in column phases so q-tile 0's
            # columns (both halves' first 512) land first
            nc.sync.dma_start(out=w_all[:], in_=W[:])
            nc.sync.dma_start(out=m_all[:], in_=mask[:])
            # PE pre-warm during the input-DMA wait: HAM un-throttles after
            # ~3.4us of sustained activity, so the first real matmuls run at
            # 2.4GHz instead of 1.2GHz
            warm_sb = persist.tile([128, 512], f16, tag="warm")
            nc.vector.memset(warm_sb[:], 0.0)
            psw = ps_sml.tile([128, 512], f32, tag="sml", name="warm_ps")
            for _ in range(8):
                nc.tensor.matmul(
                    psw[:], lhsT=warm_sb[:, 0:128], rhs=warm_sb[:],
                    start=True, stop=True,
                )
            half = S // 2
            phase_cols = [(0, 512), (512, 1024), (1024, half)]
            for pi, (lo, hi) in enumerate(phase_cols):
                bsl = [(0, 1), (1, 2)] if pi == 0 else [(0, 2)]
                for (b0, b1) in bsl:
                    for di in range(N_DIN):
                        src_v = xT[128 * di:128 * (di + 1), :].rearrange(
                            "p (b c) -> p b c", b=2)
                        dst_v = x_sb[di].rearrange("p (b c) -> p b c", b=2)
                        nc.gpsimd.dma_start(
                            out=dst_v[:, b0:b1, lo:hi],
                            in_=src_v[:, b0:b1, lo:hi],
                        )

            def project_kt(c):
                ps = ps_sml.tile([128, 512], f32, tag="sml", name=f"pkt{c}")
                for di in range(N_DIN):
                    nc.tensor.matmul(
                        ps[:],
                        lhsT=w_sb[di][:, D:2 * D],
                        rhs=x_sb[di][:, 512 * c:512 * (c + 1)],
                        start=(di == 0),
                        stop=(di == N_DIN - 1),
                    )
                nc.scalar.activation(kt_sb[c][:], ps[:], AF.Copy)

            def project_qt(t):
                ps = ps_sml.tile([128, 512], f32, tag="sml", name=f"pqt{t}")
                for di in range(N_DIN):
                    nc.tensor.matmul(
                        ps[:],
                        lhsT=w_sb[di][:, 0:D],
                        rhs=x_sb[di][:, TQ * t:TQ * (t + 1)],
                        start=(di == 0),
                        stop=(di == N_DIN - 1),
                    )
                nc.scalar.activation(qt_sb[t][:], ps[:], AF.Copy)

            def project_v_chunk(c):
                for k in range(4 * c, 4 * c + 4):
                    ps = ps_sml.tile([128, D], f32, tag="sml", name=f"pv{k}")
                    for di in range(N_DIN):
                        nc.tensor.matmul(
                            ps[:],
                            lhsT=x_sb[di][:, 128 * k:128 * (k + 1)],
                            rhs=w_sb[di][:, 2 * D:3 * D],
                            start=(di == 0),
                            stop=(di == N_DIN - 1),
                        )
                    nc.vector.tensor_copy(v_sb[k][:], ps[:])

            for t in range(n_qt):
                project_kt(t)
                project_kt(n_qt + t)
                project_v_chunk(t)
                project_v_chunk(n_qt + t)
                project_qt(t)

                po = ps_out.tile([128, TQ], f32, tag="out", name=f"po{t}")
                sacc = sacc_p.tile([128, 2 * TQ], f16, tag="sacc", name=f"sacc{t}")
                pairs = [2 * j for j in range(2 * (t + 1))] + [
                    n_kt_half + 2 * j for j in range(2 * (t + 1))
                ]
                n_av = 4 * (t + 1) * 2
                for i, kp in enumerate(pairs):
                    ps = ps_big.tile([128, 2 * TQ], f32, tag="big",
                                     name=f"s{t}_{kp}")
                    pt = work.tile([128, 2 * TQ], f16, tag="pt",
                                   name=f"p{t}_{kp}")
                    half2 = kp >= n_kt_half
                    rel = kp - n_kt_half if half2 else kp
                    diag = 4 * t <= rel < 4 * t + 4
                    for s_ in (0, 1):
                        kt = kp + s_
                        nc.tensor.matmul(
                            ps[:, TQ * s_:TQ * (s_ + 1)],
                            lhsT=kt_sb[kt // 4][:, 128 * (kt % 4):128 * (kt % 4 + 1)],
                            rhs=qt_sb[t][:],
                            start=True,
                            stop=not diag,
                        )
                        if diag:
                            r0 = (4 if half2 else 0) + rel - 4 * t + s_
                            nc.tensor.matmul(
                                ps[:, TQ * s_:TQ * (s_ + 1)],
                                lhsT=ident,
                                rhs=m_sb[r0][:],
                                start=False,
                                stop=True,
                            )
                    nc.scalar.activation(pt[:], ps[:], AF.Exp, scale=float(SCALE))
                    for s_ in (0, 1):
                        kt = kp + s_
                        lo_q = 128 * (rel - 4 * t + s_) if diag else 0
                        nc.tensor.matmul(
                            po[:, lo_q:TQ],
                            lhsT=v_sb[kt][:],
                            rhs=pt[:, TQ * s_ + lo_q:TQ * (s_ + 1)],
                            start=(2 * i + s_ == 0),
                            stop=(2 * i + s_ == n_av - 1),
                        )
                    if i == 0:
                        nc.vector.tensor_copy(sacc[:], pt[:])
                    else:
                        nc.vector.tensor_add(sacc[:], sacc[:], pt[:])
                ob = outp.tile([128, TQ], f32, tag="ob", name=f"ob{t}")
                nc.scalar.activation(ob[:], po[:], AF.Copy)
                nc.sync.dma_start(out=out_num[:, TQ * t:TQ * (t + 1)], in_=ob[:])
                nc.sync.dma_start(
                    out=out_den[:, 2 * TQ * t:2 * TQ * (t + 1)], in_=sacc[:]
                )
    _split_waits(nc)
    return nc


_NC_CACHE = []


def _get_nc():
    if not _NC_CACHE:
        _NC_CACHE.append(_build())
    return _NC_CACHE[0]


def _host_inputs(x, Wq, Wk, Wv):
    W3 = np.concatenate([Wq, Wk, Wv], axis=1).astype(np.float16)  # [768, 384]
    W = np.ascontiguousarray(
        W3.reshape(N_DIN, 128, 3 * D).transpose(1, 0, 2).reshape(128, N_DIN * 3 * D)
    )
    u = np.arange(128)[:, None]
    i = np.arange(TQ)[None, :]
    masks = {}
    for h in (0, 1):
        m = np.zeros((8, 128, TQ), np.float32)
        for r in range(4):
            m[r] = (128 * r + u <= i)
            m[4 + r] = (128 * r + u <= i - 1 + h)
        ma = (m - 1.0) * 1000.0  # 0 allowed, -1000 disallowed
        flat = ma.transpose(1, 0, 2).reshape(128, 8 * TQ)
        masks[h] = np.ascontiguousarray(
            np.concatenate([flat, np.eye(128, dtype=np.float32)], axis=1)
        ).astype(np.float16)
    in_maps = []
    for c in range(2 * B):
        b, h = divmod(c, 2)
        xp = np.concatenate([x[b, h::2], x[b, 1 - h::2]], axis=0)  # [S, 768]
        xT_p = np.ascontiguousarray(xp.T.astype(np.float16))  # [768, S]
        in_maps.append({"xT": xT_p, "W": W, "mask": masks[h]})
    return in_maps


def kernel(x, Wq, Wk, Wv):
    x = np.asarray(x, np.float32)
    Wq = np.asarray(Wq, np.float32)
    Wk = np.asarray(Wk, np.float32)
    Wv = np.asarray(Wv, np.float32)
    nc = _get_nc()
    in_maps = _host_inputs(x, Wq, Wk, Wv)
    res = run_bass_kernel_spmd(nc, in_maps, list(range(2 * B)))
    out = np.empty((B, S, D), np.float32)
    NQ = S // 2
    for c in range(2 * B):
        b, h = divmod(c, 2)
        num = res.results[c]["out_num"]  # [128, NQ] f32
        sacc = res.results[c]["out_den"].astype(np.float32)  # [128, 2*NQ]
        s3 = sacc.reshape(128, NQ // TQ, 2, TQ)
        den = s3.sum(axis=(0, 2)).reshape(NQ)
        out[b, h::2, :] = (num / den[None, :]).T
    return out



# revision 2
# speedup vs baseline: 1.0028x; 1.0028x over previous
"""Causal-attention (QKV projection + softmax(QK^T/sqrt(d))V) on 8 trn2 cores.

Contract: kernel(x, Wq, Wk, Wv) takes FULL inputs
  x [4, 4096, 768] f32, Wq/Wk/Wv [768, 128] f32
and returns the FULL output [4, 4096, 128] f32.

Sharding: 2 cores per batch. Core with parity h in {0,1} of batch b owns query
rows h::2 (perfect causal load balance). The host permutes the per-core input
to xT_p = concat(x[b, h::2], x[b, 1-h::2]).T so one compiled SPMD program runs
on every core.

Per-core device program (fp16 matmuls, fp32 PSUM accumulation):
  K^T[d=128, S], Q^T[d=128, S/2], V[key-tile][128 keys, 128 d] projections;
  per 512-query tile: scores^T tiles [128 keys, 512 q] -> exp on ScalarE
  (no max subtraction: scores ~ N(0,1)). Causality of the permuted key order
  reduces to a [128,128] triangular wedge per diagonal key-tile, applied as a
  multiplicative 0/1 mask on VectorE after the exp; score matmuls / exp / sum
  accumulation are column-trimmed on diagonal tiles.
  Outputs: numerator OUT^T [128, S/2] f32 and exp-sum tiles [128, 2*S/2] f16;
  the host reduces the exp-sums to denominators, divides, and scatters.
"""
import numpy as np

import concourse.bass as bass
import concourse.mybir as mybir
import concourse.tile as tile_mod
from concourse.tile import ScopedClock, VectorClock
from concourse.tile_sem_assignment import N_PROCS
from concourse.bass_utils import run_bass_kernel_spmd

f32 = mybir.dt.float32
f16 = mybir.dt.float16

B, S, D_IN, D = 4, 4096, 768, 128
N_DIN = D_IN // 128  # 6
TQ = 512             # queries per q-tile
SCALE = 1.0 / np.sqrt(np.float32(D))
AF = mybir.ActivationFunctionType

# ---------------------------------------------------------------------------
# Workarounds: the walrus build in this container accepts only ONE sync-wait
# command per instruction. TileContext's exit drain carries one wait per
# active proc, and Tile's sem assignment emits multi-wait instructions.
# Split both onto single-wait carrier instructions.
# ---------------------------------------------------------------------------


def _split_drain_and_barrier(self, tick_clock, wait_clock):
    gc = tick_clock.global_clock
    for p in range(N_PROCS):
        if gc[p] == 0:
            continue
        vc = VectorClock([gc[q] if q == p else 0 for q in range(N_PROCS)])
        d = self.nc.sync.drain()
        wait_clock.add_sem_waits(d.ins, ScopedClock({None: vc}))
    self.nc.all_engine_barrier()
    assert self.sems is not None
    popped = self.nc._tile_sem_poison_stack.pop()
    assert popped is self._sem_poison
    self.nc.clear_and_free_semaphores(list(self.sems.allocated().values()))
    self.nc.all_engine_barrier()


tile_mod.TileContext._drain_and_barrier = _split_drain_and_barrier


def _split_waits(nc, max_waits=1):
    for fn in nc.m.functions:
        for bb in fn.blocks:
            insts = bb.instructions
            if not any(
                i.sync_info and i.sync_info.on_wait
                and len(i.sync_info.on_wait) > max_waits
                for i in insts
            ):
                continue
            new = []
            for inst in insts:
                si = inst.sync_info
                ow = list(si.on_wait) if si and si.on_wait else []
                if len(ow) > max_waits:
                    excess, keep = ow[:-max_waits], ow[-max_waits:]
                    for j, w in enumerate(excess):
                        new.append(
                            mybir.InstEventSemaphore(
                                name=f"{inst.name}-wsplit{j}",
                                engine=inst.engine,
                                ins=[],
                                outs=[],
                                sync_info=mybir.SyncInfo(
                                    on_wait=[w], on_update=[]
                                ),
                            )
                        )
                    inst.sync_info = mybir.SyncInfo(
                        on_wait=keep, on_update=list(si.on_update or [])
                    )
                new.append(inst)
            bb.instructions = new


# ---------------------------------------------------------------------------
# Device program
# ---------------------------------------------------------------------------


def _build():
    NQ = S // 2
    n_qt = NQ // TQ          # 4
    n_kt_half = NQ // 128    # 16
    half = S // 2

    nc = bass.Bass()
    xT = nc.declare_dram_parameter("xT", [D_IN, S], f16, isOutput=False)
    W = nc.declare_dram_parameter("W", [128, N_DIN * 3 * D], f16, isOutput=False)
    mask = nc.declare_dram_parameter("mask", [128, 256], f16, isOutput=False)
    out_num = nc.declare_dram_parameter("out_num", [D, NQ], f32, isOutput=True)
    out_den = nc.declare_dram_parameter("out_den", [128, 2 * NQ], f16, isOutput=True)

    with tile_mod.TileContext(nc) as tc:
        with (
            tc.tile_pool(name="persist", bufs=1) as persist,
            tc.tile_pool(name="work", bufs=8) as work,
            tc.tile_pool(name="sacc_p", bufs=2) as sacc_p,
            tc.tile_pool(name="outp", bufs=2) as outp,
            tc.tile_pool(name="ps_s", bufs=2, space="PSUM") as ps_s,
            tc.tile_pool(name="ps_o", bufs=2, space="PSUM") as ps_o,
            tc.tile_pool(name="ps_p", bufs=2, space="PSUM") as ps_p,
        ):
            x_sb = [persist.tile([128, S], f16, tag=f"x{di}", name=f"x{di}")
                    for di in range(N_DIN)]
            w_all = persist.tile([128, N_DIN * 3 * D], f16, tag="w_all")
            m_all = persist.tile([128, 256], f16, tag="m_all")
            kt_sb = [persist.tile([128, 512], f16, tag=f"kt{c}", name=f"kt{c}")
                     for c in range(S // 512)]
            qt_sb = [persist.tile([128, TQ], f16, tag=f"qt{t}", name=f"qt{t}")
                     for t in range(n_qt)]
            v_sb = [persist.tile([128, D], f16, tag=f"v{k}", name=f"v{k}")
                    for k in range(2 * n_kt_half)]

            w_sb = [w_all[:, 3 * D * di:3 * D * (di + 1)] for di in range(N_DIN)]

            # --- input DMAs -------------------------------------------------
            # Trigger issue is ~650ns per dma_start per queue, so spread the
            # critical first columns across three queues: half-1 cols 0:512 on
            # the Scalar HWDGE queue, W + mask + half-2 cols 0:512 on Sync,
            # and the remaining column phases on the GpSimd SWDGE queue.
            for di in range(N_DIN):
                nc.scalar.dma_start(
                    out=x_sb[di][:, 0:512],
                    in_=xT[128 * di:128 * (di + 1), 0:512],
                )
            nc.sync.dma_start(out=w_all[:], in_=W[:])
            nc.sync.dma_start(out=m_all[:], in_=mask[:])
            for di in range(N_DIN):
                nc.sync.dma_start(
                    out=x_sb[di][:, half:half + 512],
                    in_=xT[128 * di:128 * (di + 1), half:half + 512],
                )
            for lo, hi in ((512, 1024), (1024, half)):
                for di in range(N_DIN):
                    src_v = xT[128 * di:128 * (di + 1), :].rearrange(
                        "p (b c) -> p b c", b=2)
                    dst_v = x_sb[di].rearrange("p (b c) -> p b c", b=2)
                    nc.gpsimd.dma_start(
                        out=dst_v[:, 0:2, lo:hi],
                        in_=src_v[:, 0:2, lo:hi],
                    )

            # PE pre-warm during the input-DMA wait: HAM un-throttles after
            # ~3.4us of sustained activity, so the first real matmuls run at
            # 2.4GHz instead of 1.2GHz
            warm_sb = persist.tile([128, 512], f16, tag="warm")
            nc.vector.memset(warm_sb[:], 0.0)
            psw = ps_p.tile([128, 512], f32, tag="p", name="warm_ps")
            for _ in range(8):
                nc.tensor.matmul(
                    psw[:], lhsT=warm_sb[:, 0:128], rhs=warm_sb[:],
                    start=True, stop=True,
                )

            def project_kt(c):
                ps = ps_p.tile([128, 512], f32, tag="p", name=f"pkt{c}")
                for di in range(N_DIN):
                    nc.tensor.matmul(
                        ps[:],
                        lhsT=w_sb[di][:, D:2 * D],
                        rhs=x_sb[di][:, 512 * c:512 * (c + 1)],
                        start=(di == 0),
                        stop=(di == N_DIN - 1),
                    )
                nc.vector.tensor_copy(kt_sb[c][:], ps[:])

            def project_qt(t):
                ps = ps_p.tile([128, 512], f32, tag="p", name=f"pqt{t}")
                for di in range(N_DIN):
                    nc.tensor.matmul(
                        ps[:],
                        lhsT=w_sb[di][:, 0:D],
                        rhs=x_sb[di][:, TQ * t:TQ * (t + 1)],
                        start=(di == 0),
                        stop=(di == N_DIN - 1),
                    )
                nc.scalar.activation(qt_sb[t][:], ps[:], AF.Copy)

            def project_v_chunk(c):
                for k in range(4 * c, 4 * c + 4):
                    ps = ps_p.tile([128, D], f32, tag="p", name=f"pv{k}")
                    for di in range(N_DIN):
                        nc.tensor.matmul(
                            ps[:],
                            lhsT=x_sb[di][:, 128 * k:128 * (k + 1)],
                            rhs=w_sb[di][:, 2 * D:3 * D],
                            start=(di == 0),
                            stop=(di == N_DIN - 1),
                        )
                    nc.vector.tensor_copy(v_sb[k][:], ps[:])

            for t in range(n_qt):
                project_kt(t)
                project_kt(n_qt + t)
                project_v_chunk(t)
                project_v_chunk(n_qt + t)
                project_qt(t)

                po = ps_o.tile([128, TQ], f32, tag="o", name=f"po{t}")
                sacc = sacc_p.tile([128, 2 * TQ], f16, tag="sacc", name=f"sacc{t}")
                nc.vector.memset(sacc[:], 0.0)
                pairs = [2 * j for j in range(2 * (t + 1))] + [
                    n_kt_half + 2 * j for j in range(2 * (t + 1))
                ]
                n_av = 4 * (t + 1) * 2
                for i, kp in enumerate(pairs):
                    ps = ps_s.tile([128, 2 * TQ], f32, tag="s",
                                   name=f"s{t}_{kp}")
                    pt = work.tile([128, 2 * TQ], f16, tag="pt",
                                   name=f"p{t}_{kp}")
                    half2 = kp >= n_kt_half
                    rel = kp - n_kt_half if half2 else kp
                    diag = 4 * t <= rel < 4 * t + 4
                    los = []
                    for s_ in (0, 1):
                        kt = kp + s_
                        lo = 128 * (rel + s_ - 4 * t) if diag else 0
                        los.append(lo)
                        nc.tensor.matmul(
                            ps[:, TQ * s_ + lo:TQ * (s_ + 1)],
                            lhsT=kt_sb[kt // 4][:, 128 * (kt % 4):128 * (kt % 4 + 1)],
                            rhs=qt_sb[t][:, lo:TQ],
                            start=True,
                            stop=True,
                        )
                    if diag:
                        wm = m_all[:, 128:256] if half2 else m_all[:, 0:128]
                        for s_ in (0, 1):
                            lo = los[s_]
                            nc.scalar.activation(
                                pt[:, TQ * s_ + lo:TQ * (s_ + 1)],
                                ps[:, TQ * s_ + lo:TQ * (s_ + 1)],
                                AF.Exp, scale=float(SCALE),
                            )
                            # zero the disallowed triangular wedge
                            nc.vector.tensor_mul(
                                pt[:, TQ * s_ + lo:TQ * s_ + lo + 128],
                                pt[:, TQ * s_ + lo:TQ * s_ + lo + 128],
                                wm,
                            )
                            nc.vector.tensor_add(
                                sacc[:, TQ * s_ + lo:TQ * (s_ + 1)],
                                sacc[:, TQ * s_ + lo:TQ * (s_ + 1)],
                                pt[:, TQ * s_ + lo:TQ * (s_ + 1)],
                            )
                    else:
                        nc.scalar.activation(pt[:], ps[:], AF.Exp,
                                             scale=float(SCALE))
                        nc.vector.tensor_add(sacc[:], sacc[:], pt[:])
                    for s_ in (0, 1):
                        kt = kp + s_
                        lo = los[s_]
                        nc.tensor.matmul(
                            po[:, lo:TQ],
                            lhsT=v_sb[kt][:],
                            rhs=pt[:, TQ * s_ + lo:TQ * (s_ + 1)],
                            start=(2 * i + s_ == 0),
                            stop=(2 * i + s_ == n_av - 1),
                        )
                ob = outp.tile([128, TQ], f32, tag="ob", name=f"ob{t}")
                nc.scalar.activation(ob[:], po[:], AF.Copy)
                nc.sync.dma_start(out=out_num[:, TQ * t:TQ * (t + 1)], in_=ob[:])
                nc.sync.dma_start(
                    out=out_den[:, 2 * TQ * t:2 * TQ * (t + 1)], in_=sacc[:]
                )
    _split_waits(nc)
    return nc


_NC_CACHE = []


def _get_nc():
    if not _NC_CACHE:
        _NC_CACHE.append(_build())
    return _NC_CACHE[0]


def _host_inputs(x, Wq, Wk, Wv):
    W3 = np.concatenate([Wq, Wk, Wv], axis=1).astype(np.float16)  # [768, 384]
    W = np.ascontiguousarray(
        W3.reshape(N_DIN, 128, 3 * D).transpose(1, 0, 2).reshape(128, N_DIN * 3 * D)
    )
    u = np.arange(128)[:, None]
    c = np.arange(128)[None, :]
    masks = {}
    for h in (0, 1):
        w0 = (u <= c).astype(np.float16)          # first-half wedge
        w1 = (u <= c - 1 + h).astype(np.float16)  # second-half wedge
        masks[h] = np.ascontiguousarray(np.concatenate([w0, w1], axis=1))
    in_maps = []
    for core in range(2 * B):
        b, h = divmod(core, 2)
        xp = np.concatenate([x[b, h::2], x[b, 1 - h::2]], axis=0)  # [S, 768]
        xT_p = np.ascontiguousarray(xp.T.astype(np.float16))  # [768, S]
        in_maps.append({"xT": xT_p, "W": W, "mask": masks[h]})
    return in_maps


def kernel(x, Wq, Wk, Wv):
    x = np.asarray(x, np.float32)
    Wq = np.asarray(Wq, np.float32)
    Wk = np.asarray(Wk, np.float32)
    Wv = np.asarray(Wv, np.float32)
    nc = _get_nc()
    in_maps = _host_inputs(x, Wq, Wk, Wv)
    res = run_bass_kernel_spmd(nc, in_maps, list(range(2 * B)))
    out = np.empty((B, S, D), np.float32)
    NQ = S // 2
    for core in range(2 * B):
        b, h = divmod(core, 2)
        num = res.results[core]["out_num"]  # [128, NQ] f32
        sacc = res.results[core]["out_den"].astype(np.float32)  # [128, 2*NQ]
        s3 = sacc.reshape(128, NQ // TQ, 2, TQ)
        den = s3.sum(axis=(0, 2)).reshape(NQ)
        out[b, h::2, :] = (num / den[None, :]).T
    return out


# revision 6
# speedup vs baseline: 1.0784x; 1.0754x over previous
"""Causal-attention (QKV projection + softmax(QK^T/sqrt(d))V) on 8 trn2 cores.

Contract: kernel(x, Wq, Wk, Wv) takes FULL inputs
  x [4, 4096, 768] f32, Wq/Wk/Wv [768, 128] f32
and returns the FULL output [4, 4096, 128] f32.

Sharding: 2 cores per batch. Core with parity h in {0,1} of batch b owns query
rows h::2 (perfect causal load balance). The host permutes the per-core input
to xT_p = concat(x[b, h::2], x[b, 1-h::2]).T so one compiled SPMD program runs
on every core.

Per-core device program (fp16 matmuls, fp32 PSUM accumulation):
  K^T[d=128, S], Q^T[d=128, S/2], V[key-tile][128 keys, 128 d] projections;
  per 512-query tile: scores^T tiles [128 keys, 512 q] -> exp on ScalarE
  (no max subtraction: scores ~ N(0,1)). Causality of the permuted key order
  reduces to a [128,128] triangular wedge per diagonal key-tile, applied as a
  multiplicative 0/1 mask on VectorE after the exp; score matmuls / exp / sum
  accumulation are column-trimmed on diagonal tiles.
  Outputs: numerator OUT^T [128, S/2] f32 and exp-sum tiles [128, 2*S/2] f16;
  the host reduces the exp-sums to denominators, divides, and scatters.
"""
import numpy as np

import concourse.bass as bass
import concourse.mybir as mybir
import concourse.tile as tile_mod
from concourse.tile import ScopedClock, VectorClock
from concourse.tile_sem_assignment import N_PROCS
from concourse.bass_utils import run_bass_kernel_spmd

f32 = mybir.dt.float32
f16 = mybir.dt.float16

B, S, D_IN, D = 4, 4096, 768, 128
N_DIN = D_IN // 128  # 6
TQ = 512             # queries per q-tile
SCALE = 1.0 / np.sqrt(np.float32(D))
AF = mybir.ActivationFunctionType

# ---------------------------------------------------------------------------
# Workarounds: the walrus build in this container accepts only ONE sync-wait
# command per instruction. TileContext's exit drain carries one wait per
# active proc, and Tile's sem assignment emits multi-wait instructions.
# Split both onto single-wait carrier instructions.
# ---------------------------------------------------------------------------


def _split_drain_and_barrier(self, tick_clock, wait_clock):
    gc = tick_clock.global_clock
    for p in range(N_PROCS):
        if gc[p] == 0:
            continue
        vc = VectorClock([gc[q] if q == p else 0 for q in range(N_PROCS)])
        d = self.nc.sync.drain()
        wait_clock.add_sem_waits(d.ins, ScopedClock({None: vc}))
    self.nc.all_engine_barrier()
    assert self.sems is not None
    popped = self.nc._tile_sem_poison_stack.pop()
    assert popped is self._sem_poison
    self.nc.clear_and_free_semaphores(list(self.sems.allocated().values()))
    self.nc.all_engine_barrier()


tile_mod.TileContext._drain_and_barrier = _split_drain_and_barrier


def _split_waits(nc, max_waits=1):
    for fn in nc.m.functions:
        for bb in fn.blocks:
            insts = bb.instructions
            if not any(
                i.sync_info and i.sync_info.on_wait
                and len(i.sync_info.on_wait) > max_waits
                for i in insts
            ):
                continue
            new = []
            for inst in insts:
                si = inst.sync_info
                ow = list(si.on_wait) if si and si.on_wait else []
                if len(ow) > max_waits:
                    excess, keep = ow[:-max_waits], ow[-max_waits:]
                    for j, w in enumerate(excess):
                        new.append(
                            mybir.InstEventSemaphore(
                                name=f"{inst.name}-wsplit{j}",
                                engine=inst.engine,
                                ins=[],
                                outs=[],
                                sync_info=mybir.SyncInfo(
                                    on_wait=[w], on_update=[]
                                ),
                            )
                        )
                    inst.sync_info = mybir.SyncInfo(
                        on_wait=keep, on_update=list(si.on_update or [])
                    )
                new.append(inst)
            bb.instructions = new


# ---------------------------------------------------------------------------
# Device program
# ---------------------------------------------------------------------------


def _build():
    NQ = S // 2
    n_qt = NQ // TQ          # 4
    n_kt_half = NQ // 128    # 16
    half = S // 2

    nc = bass.Bass()
    xT = nc.declare_dram_parameter("xT", [D_IN, S], f16, isOutput=False)
    W = nc.declare_dram_parameter("W", [128, N_DIN * 3 * D], f16, isOutput=False)
    mask = nc.declare_dram_parameter("mask", [128, 256], f16, isOutput=False)
    out_num = nc.declare_dram_parameter("out_num", [D, NQ], f32, isOutput=True)
    out_den = nc.declare_dram_parameter("out_den", [128, 2 * NQ], f16, isOutput=True)

    with tile_mod.TileContext(nc) as tc:
        with (
            tc.tile_pool(name="persist", bufs=1) as persist,
            tc.tile_pool(name="work", bufs=8) as work,
            tc.tile_pool(name="sacc_p", bufs=2) as sacc_p,
            tc.tile_pool(name="outp", bufs=2) as outp,
            tc.tile_pool(name="ps_s", bufs=2, space="PSUM") as ps_s,
            tc.tile_pool(name="ps_o", bufs=2, space="PSUM") as ps_o,
            tc.tile_pool(name="ps_p", bufs=2, space="PSUM") as ps_p,
        ):
            x_all = persist.tile([128, N_DIN * S], f16, tag="x_all")
            x_sb = [x_all[:, S * di:S * (di + 1)] for di in range(N_DIN)]
            w_all = persist.tile([128, N_DIN * 3 * D], f16, tag="w_all")
            m_all = persist.tile([128, 256], f16, tag="m_all")
            kt_sb = [persist.tile([128, 512], f16, tag=f"kt{c}", name=f"kt{c}")
                     for c in range(S // 512)]
            qt_sb = [persist.tile([128, TQ], f16, tag=f"qt{t}", name=f"qt{t}")
                     for t in range(n_qt)]
            v_sb = [persist.tile([128, D], f16, tag=f"v{k}", name=f"v{k}")
                    for k in range(2 * n_kt_half)]

            w_sb = [w_all[:, 3 * D * di:3 * D * (di + 1)] for di in range(N_DIN)]

            # --- input DMAs -------------------------------------------------
            # The GpSimd SWDGE queue sustains ~300GB/s (vs ~60GB/s for the
            # Sync/Scalar HWDGE queues), so ALL bulk input goes there, ordered
            # by first use, with few big multi-dim triggers (issue is ~750ns
            # per dma_start): W, then x in column phases.
            x3s = xT.rearrange("(d p) c -> p d c", p=128)
            x3d = x_all.rearrange("p (d c) -> p d c", d=N_DIN)
            nc.gpsimd.dma_start(out=w_all[:], in_=W[:])
            nc.gpsimd.dma_start(out=x3d[:, :, 0:512], in_=x3s[:, :, 0:512])
            nc.gpsimd.dma_start(
                out=x3d[:, :, half:half + 512], in_=x3s[:, :, half:half + 512]
            )
            for lo, hi in ((512, 1024), (1024, half)):
                for off in (0, half):
                    nc.gpsimd.dma_start(
                        out=x3d[:, :, off + lo:off + hi],
                        in_=x3s[:, :, off + lo:off + hi],
                    )
            nc.sync.dma_start(out=m_all[:], in_=mask[:])

            # PE pre-warm during the input-DMA wait: HAM un-throttles after
            # ~3.4us of sustained activity, so the first real matmuls run at
            # 2.4GHz instead of 1.2GHz
            warm_sb = persist.tile([128, 512], f16, tag="warm")
            nc.vector.memset(warm_sb[:], 0.0)
            psw = ps_p.tile([128, 512], f32, tag="p", name="warm_ps")
            for _ in range(13):
                nc.tensor.matmul(
                    psw[:], lhsT=warm_sb[:, 0:128], rhs=warm_sb[:],
                    start=True, stop=True,
                )

            def project_kt(c):
                ps = ps_p.tile([128, 512], f32, tag="p", name=f"pkt{c}")
                for di in range(N_DIN):
                    nc.tensor.matmul(
                        ps[:],
                        lhsT=w_sb[di][:, D:2 * D],
                        rhs=x_sb[di][:, 512 * c:512 * (c + 1)],
                        start=(di == 0),
                        stop=(di == N_DIN - 1),
                    )
                nc.vector.tensor_copy(kt_sb[c][:], ps[:])

            def project_qt(t):
                ps = ps_p.tile([128, 512], f32, tag="p", name=f"pqt{t}")
                for di in range(N_DIN):
                    nc.tensor.matmul(
                        ps[:],
                        lhsT=w_sb[di][:, 0:D],
                        rhs=x_sb[di][:, TQ * t:TQ * (t + 1)],
                        start=(di == 0),
                        stop=(di == N_DIN - 1),
                    )
                nc.scalar.activation(qt_sb[t][:], ps[:], AF.Copy)

            def project_v_chunk(c):
                for k in range(4 * c, 4 * c + 4):
                    ps = ps_p.tile([128, D], f32, tag="p", name=f"pv{k}")
                    for di in range(N_DIN):
                        nc.tensor.matmul(
                            ps[:],
                            lhsT=x_sb[di][:, 128 * k:128 * (k + 1)],
                            rhs=w_sb[di][:, 2 * D:3 * D],
                            start=(di == 0),
                            stop=(di == N_DIN - 1),
                        )
                    nc.vector.tensor_copy(v_sb[k][:], ps[:])

            def emit_pair(t, i, kp, po, sacc, n_av):
                ps = ps_s.tile([128, 2 * TQ], f32, tag="s", name=f"s{t}_{kp}")
                pt = work.tile([128, 2 * TQ], f16, tag="pt", name=f"p{t}_{kp}")
                half2 = kp >= n_kt_half
                rel = kp - n_kt_half if half2 else kp
                diag = 4 * t <= rel < 4 * t + 4
                los = []
                for s_ in (0, 1):
                    kt = kp + s_
                    lo = 128 * (rel + s_ - 4 * t) if diag else 0
                    los.append(lo)
                    nc.tensor.matmul(
                        ps[:, TQ * s_ + lo:TQ * (s_ + 1)],
                        lhsT=kt_sb[kt // 4][:, 128 * (kt % 4):128 * (kt % 4 + 1)],
                        rhs=qt_sb[t][:, lo:TQ],
                        start=True,
                        stop=True,
                    )
                if diag:
                    wm = m_all[:, 128:256] if half2 else m_all[:, 0:128]
                    for s_ in (0, 1):
                        lo = los[s_]
                        nc.scalar.activation(
                            pt[:, TQ * s_ + lo:TQ * (s_ + 1)],
                            ps[:, TQ * s_ + lo:TQ * (s_ + 1)],
                            AF.Exp, scale=float(SCALE),
                        )
                        # zero the disallowed triangular wedge
                        nc.vector.tensor_mul(
                            pt[:, TQ * s_ + lo:TQ * s_ + lo + 128],
                            pt[:, TQ * s_ + lo:TQ * s_ + lo + 128],
                            wm,
                        )
                        nc.vector.tensor_add(
                            sacc[:, TQ * s_ + lo:TQ * (s_ + 1)],
                            sacc[:, TQ * s_ + lo:TQ * (s_ + 1)],
                            pt[:, TQ * s_ + lo:TQ * (s_ + 1)],
                        )
                else:
                    nc.scalar.activation(pt[:], ps[:], AF.Exp,
                                         scale=float(SCALE))
                    nc.vector.tensor_add(sacc[:], sacc[:], pt[:])
                for s_ in (0, 1):
                    kt = kp + s_
                    lo = los[s_]
                    nc.tensor.matmul(
                        po[:, lo:TQ],
                        lhsT=v_sb[kt][:],
                        rhs=pt[:, TQ * s_ + lo:TQ * (s_ + 1)],
                        start=(2 * i + s_ == 0),
                        stop=(2 * i + s_ == n_av - 1),
                    )

            for t in range(n_qt):
                po = ps_o.tile([128, TQ], f32, tag="o", name=f"po{t}")
                sacc = sacc_p.tile([128, 2 * TQ], f16, tag="sacc", name=f"sacc{t}")
                pairs = [2 * j for j in range(2 * (t + 1))] + [
                    n_kt_half + 2 * j for j in range(2 * (t + 1))
                ]
                n_av = 4 * (t + 1) * 2
                if t == 0:
                    # half-2 x data lands later than half-1: process the
                    # first-half pairs before touching half-2 projections
                    project_kt(0)
                    project_v_chunk(0)
                    project_qt(0)
                    nc.vector.memset(sacc[:], 0.0)
                    emit_pair(t, 0, pairs[0], po, sacc, n_av)
                    emit_pair(t, 1, pairs[1], po, sacc, n_av)
                    project_kt(n_qt)
                    project_v_chunk(n_qt)
                    emit_pair(t, 2, pairs[2], po, sacc, n_av)
                    emit_pair(t, 3, pairs[3], po, sacc, n_av)
                else:
                    project_kt(t)
                    project_kt(n_qt + t)
                    project_v_chunk(t)
                    project_v_chunk(n_qt + t)
                    project_qt(t)
                    nc.vector.memset(sacc[:], 0.0)
                    for i, kp in enumerate(pairs):
                        emit_pair(t, i, kp, po, sacc, n_av)
                ob = outp.tile([128, TQ], f32, tag="ob", name=f"ob{t}")
                nc.scalar.activation(ob[:], po[:], AF.Copy)
                nc.sync.dma_start(out=out_num[:, TQ * t:TQ * (t + 1)], in_=ob[:])
                nc.sync.dma_start(
                    out=out_den[:, 2 * TQ * t:2 * TQ * (t + 1)], in_=sacc[:]
                )
    _split_waits(nc)
    return nc


_NC_CACHE = []


def _get_nc():
    if not _NC_CACHE:
        _NC_CACHE.append(_build())
    return _NC_CACHE[0]


def _host_inputs(x, Wq, Wk, Wv):
    W3 = np.concatenate([Wq, Wk, Wv], axis=1).astype(np.float16)  # [768, 384]
    W = np.ascontiguousarray(
        W3.reshape(N_DIN, 128, 3 * D).transpose(1, 0, 2).reshape(128, N_DIN * 3 * D)
    )
    u = np.arange(128)[:, None]
    c = np.arange(128)[None, :]
    masks = {}
    for h in (0, 1):
        w0 = (u <= c).astype(np.float16)          # first-half wedge
        w1 = (u <= c - 1 + h).astype(np.float16)  # second-half wedge
        masks[h] = np.ascontiguousarray(np.concatenate([w0, w1], axis=1))
    in_maps = []
    for core in range(2 * B):
        b, h = divmod(core, 2)
        xp = np.concatenate([x[b, h::2], x[b, 1 - h::2]], axis=0)  # [S, 768]
        xT_p = np.ascontiguousarray(xp.T.astype(np.float16))  # [768, S]
        in_maps.append({"xT": xT_p, "W": W, "mask": masks[h]})
    return in_maps


def kernel(x, Wq, Wk, Wv):
    x = np.asarray(x, np.float32)
    Wq = np.asarray(Wq, np.float32)
    Wk = np.asarray(Wk, np.float32)
    Wv = np.asarray(Wv, np.float32)
    nc = _get_nc()
    in_maps = _host_inputs(x, Wq, Wk, Wv)
    res = run_bass_kernel_spmd(nc, in_maps, list(range(2 * B)))
    out = np.empty((B, S, D), np.float32)
    NQ = S // 2
    for core in range(2 * B):
        b, h = divmod(core, 2)
        num = res.results[core]["out_num"]  # [128, NQ] f32
        sacc = res.results[core]["out_den"].astype(np.float32)  # [128, 2*NQ]
        s3 = sacc.reshape(128, NQ // TQ, 2, TQ)
        den = s3.sum(axis=(0, 2)).reshape(NQ)
        out[b, h::2, :] = (num / den[None, :]).T
    return out


# revision 8
# speedup vs baseline: 1.1768x; 1.0912x over previous
"""Causal-attention (QKV projection + softmax(QK^T/sqrt(d))V) on 8 trn2 cores.

Contract: kernel(x, Wq, Wk, Wv) takes FULL inputs
  x [4, 4096, 768] f32, Wq/Wk/Wv [768, 128] f32
and returns the FULL output [4, 4096, 128] f32.

Sharding: 2 cores per batch. Core with parity h in {0,1} of batch b owns query
rows h::2 (perfect causal load balance). The host permutes the per-core input
to xT_p = concat(x[b, h::2], x[b, 1-h::2]).T so one compiled SPMD program runs
on every core.

Per-core device program (fp16 matmuls, fp32 PSUM accumulation):
  K^T[d=128, S], Q^T[d=128, S/2], V[key-tile][128 keys, 128 d] projections;
  per 512-query tile: scores^T tiles [128 keys, 512 q] -> exp on ScalarE
  (no max subtraction: scores ~ N(0,1)). Causality of the permuted key order
  reduces to a [128,128] triangular wedge per diagonal key-tile, applied as a
  multiplicative 0/1 mask on VectorE after the exp; score matmuls / exp / sum
  accumulation are column-trimmed on diagonal tiles.
  Outputs: numerator OUT^T [128, S/2] f32 and exp-sum tiles [128, 2*S/2] f16;
  the host reduces the exp-sums to denominators, divides, and scatters.
"""
import numpy as np

import concourse.bass as bass
import concourse.mybir as mybir
import concourse.tile as tile_mod
from concourse.tile import ScopedClock, VectorClock
from concourse.tile_sem_assignment import N_PROCS
from concourse.bass_utils import run_bass_kernel_spmd

f32 = mybir.dt.float32
f16 = mybir.dt.float16

B, S, D_IN, D = 4, 4096, 768, 128
N_DIN = D_IN // 128  # 6
TQ = 512             # queries per q-tile
SCALE = 1.0 / np.sqrt(np.float32(D))
AF = mybir.ActivationFunctionType

# ---------------------------------------------------------------------------
# Workarounds: the walrus build in this container accepts only ONE sync-wait
# command per instruction. TileContext's exit drain carries one wait per
# active proc, and Tile's sem assignment emits multi-wait instructions.
# Split both onto single-wait carrier instructions.
# ---------------------------------------------------------------------------


def _split_drain_and_barrier(self, tick_clock, wait_clock):
    gc = tick_clock.global_clock
    for p in range(N_PROCS):
        if gc[p] == 0:
            continue
        vc = VectorClock([gc[q] if q == p else 0 for q in range(N_PROCS)])
        d = self.nc.sync.drain()
        wait_clock.add_sem_waits(d.ins, ScopedClock({None: vc}))
    self.nc.all_engine_barrier()
    assert self.sems is not None
    popped = self.nc._tile_sem_poison_stack.pop()
    assert popped is self._sem_poison
    self.nc.clear_and_free_semaphores(list(self.sems.allocated().values()))
    self.nc.all_engine_barrier()


tile_mod.TileContext._drain_and_barrier = _split_drain_and_barrier


def _split_waits(nc, max_waits=1):
    for fn in nc.m.functions:
        for bb in fn.blocks:
            insts = bb.instructions
            if not any(
                i.sync_info and i.sync_info.on_wait
                and len(i.sync_info.on_wait) > max_waits
                for i in insts
            ):
                continue
            new = []
            for inst in insts:
                si = inst.sync_info
                ow = list(si.on_wait) if si and si.on_wait else []
                if len(ow) > max_waits:
                    excess, keep = ow[:-max_waits], ow[-max_waits:]
                    for j, w in enumerate(excess):
                        new.append(
                            mybir.InstEventSemaphore(
                                name=f"{inst.name}-wsplit{j}",
                                engine=inst.engine,
                                ins=[],
                                outs=[],
                                sync_info=mybir.SyncInfo(
                                    on_wait=[w], on_update=[]
                                ),
                            )
                        )
                    inst.sync_info = mybir.SyncInfo(
                        on_wait=keep, on_update=list(si.on_update or [])
                    )
                new.append(inst)
            bb.instructions = new


# ---------------------------------------------------------------------------
# Device program
# ---------------------------------------------------------------------------


def _build():
    NQ = S // 2
    n_qt = NQ // TQ          # 4
    n_kt_half = NQ // 128    # 16
    half = S // 2

    nc = bass.Bass()
    xT = nc.declare_dram_parameter("xT", [D_IN, S], f16, isOutput=False)
    W = nc.declare_dram_parameter("W", [128, N_DIN * 3 * D], f16, isOutput=False)
    mask = nc.declare_dram_parameter("mask", [128, 256], f16, isOutput=False)
    out_num = nc.declare_dram_parameter("out_num", [D, NQ], f32, isOutput=True)
    out_den = nc.declare_dram_parameter("out_den", [128, 2 * NQ], f16, isOutput=True)

    with tile_mod.TileContext(nc) as tc:
        with (
            tc.tile_pool(name="persist", bufs=1) as persist,
            tc.tile_pool(name="work", bufs=8) as work,
            tc.tile_pool(name="sacc_p", bufs=2) as sacc_p,
            tc.tile_pool(name="outp", bufs=2) as outp,
            tc.tile_pool(name="ps_s", bufs=2, space="PSUM") as ps_s,
            tc.tile_pool(name="ps_o", bufs=2, space="PSUM") as ps_o,
            tc.tile_pool(name="ps_p", bufs=2, space="PSUM") as ps_p,
        ):
            x_all = persist.tile([128, N_DIN * S], f16, tag="x_all")
            x_sb = [x_all[:, S * di:S * (di + 1)] for di in range(N_DIN)]
            w_all = persist.tile([128, N_DIN * 3 * D], f16, tag="w_all")
            m_all = persist.tile([128, 256], f16, tag="m_all")
            kt_sb = [persist.tile([128, 512], f16, tag=f"kt{c}", name=f"kt{c}")
                     for c in range(S // 512)]
            qt_sb = [persist.tile([128, TQ], f16, tag=f"qt{t}", name=f"qt{t}")
                     for t in range(n_qt)]
            v_sb = [persist.tile([128, D], f16, tag=f"v{k}", name=f"v{k}")
                    for k in range(2 * n_kt_half)]

            w_sb = [w_all[:, 3 * D * di:3 * D * (di + 1)] for di in range(N_DIN)]

            # --- input DMAs -------------------------------------------------
            # The GpSimd SWDGE queue sustains ~300GB/s (vs ~60GB/s for the
            # Sync/Scalar HWDGE queues), so ALL bulk input goes there, ordered
            # by first use, with few big multi-dim triggers (issue is ~750ns
            # per dma_start): W, then x in column phases.
            x3s = xT.rearrange("(d p) c -> p d c", p=128)
            x3d = x_all.rearrange("p (d c) -> p d c", d=N_DIN)
            nc.gpsimd.dma_start(out=w_all[:], in_=W[:])
            nc.gpsimd.dma_start(out=x3d[:, :, 0:512], in_=x3s[:, :, 0:512])
            nc.gpsimd.dma_start(
                out=x3d[:, :, half:half + 512], in_=x3s[:, :, half:half + 512]
            )
            for lo, hi in ((512, 1024), (1024, half)):
                for off in (0, half):
                    nc.gpsimd.dma_start(
                        out=x3d[:, :, off + lo:off + hi],
                        in_=x3s[:, :, off + lo:off + hi],
                    )
            nc.sync.dma_start(out=m_all[:], in_=mask[:])

            # PE pre-warm during the input-DMA wait: HAM un-throttles after
            # ~3.4us of sustained activity, so the first real matmuls run at
            # 2.4GHz instead of 1.2GHz
            warm_sb = persist.tile([128, 512], f16, tag="warm")
            nc.vector.memset(warm_sb[:], 0.0)
            psw = ps_p.tile([128, 512], f32, tag="p", name="warm_ps")
            for _ in range(22):
                nc.tensor.matmul(
                    psw[:], lhsT=warm_sb[:, 0:128], rhs=warm_sb[:],
                    start=True, stop=True,
                )

            def project_kt(c):
                ps = ps_p.tile([128, 512], f32, tag="p", name=f"pkt{c}")
                for di in range(N_DIN):
                    nc.tensor.matmul(
                        ps[:],
                        lhsT=w_sb[di][:, D:2 * D],
                        rhs=x_sb[di][:, 512 * c:512 * (c + 1)],
                        start=(di == 0),
                        stop=(di == N_DIN - 1),
                    )
                nc.vector.tensor_copy(kt_sb[c][:], ps[:])

            def project_qt(t):
                ps = ps_p.tile([128, 512], f32, tag="p", name=f"pqt{t}")
                for di in range(N_DIN):
                    nc.tensor.matmul(
                        ps[:],
                        lhsT=w_sb[di][:, 0:D],
                        rhs=x_sb[di][:, TQ * t:TQ * (t + 1)],
                        start=(di == 0),
                        stop=(di == N_DIN - 1),
                    )
                nc.scalar.activation(qt_sb[t][:], ps[:], AF.Copy)

            def project_v_chunk(c):
                for k in range(4 * c, 4 * c + 4):
                    ps = ps_p.tile([128, D], f32, tag="p", name=f"pv{k}")
                    for di in range(N_DIN):
                        nc.tensor.matmul(
                            ps[:],
                            lhsT=x_sb[di][:, 128 * k:128 * (k + 1)],
                            rhs=w_sb[di][:, 2 * D:3 * D],
                            start=(di == 0),
                            stop=(di == N_DIN - 1),
                        )
                    nc.vector.tensor_copy(v_sb[k][:], ps[:])

            def emit_pair(t, i, kp, po, sacc, n_av):
                ps = ps_s.tile([128, 2 * TQ], f32, tag="s", name=f"s{t}_{kp}")
                pt = work.tile([128, 2 * TQ], f16, tag="pt", name=f"p{t}_{kp}")
                half2 = kp >= n_kt_half
                rel = kp - n_kt_half if half2 else kp
                diag = 4 * t <= rel < 4 * t + 4
                los = []
                for s_ in (0, 1):
                    kt = kp + s_
                    lo = 128 * (rel + s_ - 4 * t) if diag else 0
                    los.append(lo)
                    nc.tensor.matmul(
                        ps[:, TQ * s_ + lo:TQ * (s_ + 1)],
                        lhsT=kt_sb[kt // 4][:, 128 * (kt % 4):128 * (kt % 4 + 1)],
                        rhs=qt_sb[t][:, lo:TQ],
                        start=True,
                        stop=True,
                    )
                if diag:
                    wm = m_all[:, 128:256] if half2 else m_all[:, 0:128]
                    for s_ in (0, 1):
                        lo = los[s_]
                        nc.scalar.activation(
                            pt[:, TQ * s_ + lo:TQ * (s_ + 1)],
                            ps[:, TQ * s_ + lo:TQ * (s_ + 1)],
                            AF.Exp, scale=float(SCALE),
                        )
                        # zero the disallowed triangular wedge
                        nc.vector.tensor_mul(
                            pt[:, TQ * s_ + lo:TQ * s_ + lo + 128],
                            pt[:, TQ * s_ + lo:TQ * s_ + lo + 128],
                            wm,
                        )
                        nc.vector.tensor_add(
                            sacc[:, TQ * s_ + lo:TQ * (s_ + 1)],
                            sacc[:, TQ * s_ + lo:TQ * (s_ + 1)],
                            pt[:, TQ * s_ + lo:TQ * (s_ + 1)],
                        )
                else:
                    nc.scalar.activation(pt[:], ps[:], AF.Exp,
                                         scale=float(SCALE))
                    nc.vector.tensor_add(sacc[:], sacc[:], pt[:])
                for s_ in (0, 1):
                    kt = kp + s_
                    lo = los[s_]
                    nc.tensor.matmul(
                        po[:, lo:TQ],
                        lhsT=v_sb[kt][:],
                        rhs=pt[:, TQ * s_ + lo:TQ * (s_ + 1)],
                        start=(2 * i + s_ == 0),
                        stop=(2 * i + s_ == n_av - 1),
                    )

            for t in range(n_qt):
                po = ps_o.tile([128, TQ], f32, tag="o", name=f"po{t}")
                sacc = sacc_p.tile([128, 2 * TQ], f16, tag="sacc", name=f"sacc{t}")
                pairs = [2 * j for j in range(2 * (t + 1))] + [
                    n_kt_half + 2 * j for j in range(2 * (t + 1))
                ]
                n_av = 4 * (t + 1) * 2
                # Each q-tile needs only qt(t) projected up front; the
                # diagonal-chunk kt/v projections are emitted mid-phase, just
                # before the pairs that consume them. That gives the PE
                # independent work while ScalarE catches up on exps, and at
                # t=0 it matches the x-column DMA arrival order.
                project_qt(t)
                nc.vector.memset(sacc[:], 0.0)
                for i, kp in enumerate(pairs):
                    half2 = kp >= n_kt_half
                    rel = kp - n_kt_half if half2 else kp
                    if rel == 4 * t:  # diag chunk needed from here on
                        project_kt(t if not half2 else n_qt + t)
                        project_v_chunk(t if not half2 else n_qt + t)
                    emit_pair(t, i, kp, po, sacc, n_av)
                ob = outp.tile([128, TQ], f32, tag="ob", name=f"ob{t}")
                nc.scalar.activation(ob[:], po[:], AF.Copy)
                nc.gpsimd.dma_start(out=out_num[:, TQ * t:TQ * (t + 1)], in_=ob[:])
                nc.gpsimd.dma_start(
                    out=out_den[:, 2 * TQ * t:2 * TQ * (t + 1)], in_=sacc[:]
                )
    _split_waits(nc)
    return nc


_NC_CACHE = []


def _get_nc():
    if not _NC_CACHE:
        _NC_CACHE.append(_build())
    return _NC_CACHE[0]


def _host_inputs(x, Wq, Wk, Wv):
    W3 = np.concatenate([Wq, Wk, Wv], axis=1).astype(np.float16)  # [768, 384]
    W = np.ascontiguousarray(
        W3.reshape(N_DIN, 128, 3 * D).transpose(1, 0, 2).reshape(128, N_DIN * 3 * D)
    )
    u = np.arange(128)[:, None]
    c = np.arange(128)[None, :]
    masks = {}
    for h in (0, 1):
        w0 = (u <= c).astype(np.float16)          # first-half wedge
        w1 = (u <= c - 1 + h).astype(np.float16)  # second-half wedge
        masks[h] = np.ascontiguousarray(np.concatenate([w0, w1], axis=1))
    in_maps = []
    for core in range(2 * B):
        b, h = divmod(core, 2)
        xp = np.concatenate([x[b, h::2], x[b, 1 - h::2]], axis=0)  # [S, 768]
        xT_p = np.ascontiguousarray(xp.T.astype(np.float16))  # [768, S]
        in_maps.append({"xT": xT_p, "W": W, "mask": masks[h]})
    return in_maps


def kernel(x, Wq, Wk, Wv):
    x = np.asarray(x, np.float32)
    Wq = np.asarray(Wq, np.float32)
    Wk = np.asarray(Wk, np.float32)
    Wv = np.asarray(Wv, np.float32)
    nc = _get_nc()
    in_maps = _host_inputs(x, Wq, Wk, Wv)
    res = run_bass_kernel_spmd(nc, in_maps, list(range(2 * B)))
    out = np.empty((B, S, D), np.float32)
    NQ = S // 2
    for core in range(2 * B):
        b, h = divmod(core, 2)
        num = res.results[core]["out_num"]  # [128, NQ] f32
        sacc = res.results[core]["out_den"].astype(np.float32)  # [128, 2*NQ]
        s3 = sacc.reshape(128, NQ // TQ, 2, TQ)
        den = s3.sum(axis=(0, 2)).reshape(NQ)
        out[b, h::2, :] = (num / den[None, :]).T
    return out


# revision 15
# speedup vs baseline: 1.2099x; 1.0282x over previous
"""Causal-attention (QKV projection + softmax(QK^T/sqrt(d))V) on 8 trn2 cores.

Contract: kernel(x, Wq, Wk, Wv) takes FULL inputs
  x [4, 4096, 768] f32, Wq/Wk/Wv [768, 128] f32
and returns the FULL output [4, 4096, 128] f32.

Sharding: 2 cores per batch. Core with parity h in {0,1} of batch b owns query
rows h::2 (perfect causal load balance). The host permutes the per-core input
to xT_p = concat(x[b, h::2], x[b, 1-h::2]).T so one compiled SPMD program runs
on every core.

Per-core device program (fp16 matmuls, fp32 PSUM accumulation):
  K^T[d=128, S], Q^T[d=128, S/2], V[key-tile][128 keys, 128 d] projections;
  per 512-query tile: scores^T tiles [128 keys, 512 q] -> exp on ScalarE
  (no max subtraction: scores ~ N(0,1)). Causality of the permuted key order
  reduces to a [128,128] triangular wedge per diagonal key-tile, applied as a
  multiplicative 0/1 mask on VectorE after the exp; score matmuls / exp / sum
  accumulation are column-trimmed on diagonal tiles.
  Outputs: numerator OUT^T [128, S/2] f32 and exp-sum tiles [128, 2*S/2] f16;
  the host reduces the exp-sums to denominators, divides, and scatters.
"""
import numpy as np

import concourse.bass as bass
import concourse.mybir as mybir
import concourse.tile as tile_mod
from concourse.tile import ScopedClock, VectorClock
from concourse.tile_sem_assignment import N_PROCS
from concourse.bass_utils import run_bass_kernel_spmd

f32 = mybir.dt.float32
f16 = mybir.dt.float16

B, S, D_IN, D = 4, 4096, 768, 128
N_DIN = D_IN // 128  # 6
TQ = 512             # queries per q-tile
SCALE = 1.0 / np.sqrt(np.float32(D))
AF = mybir.ActivationFunctionType

# ---------------------------------------------------------------------------
# Workarounds: the walrus build in this container accepts only ONE sync-wait
# command per instruction. TileContext's exit drain carries one wait per
# active proc, and Tile's sem assignment emits multi-wait instructions.
# Split both onto single-wait carrier instructions.
# ---------------------------------------------------------------------------


def _split_drain_and_barrier(self, tick_clock, wait_clock):
    gc = tick_clock.global_clock
    for p in range(N_PROCS):
        if gc[p] == 0:
            continue
        vc = VectorClock([gc[q] if q == p else 0 for q in range(N_PROCS)])
        d = self.nc.sync.drain()
        wait_clock.add_sem_waits(d.ins, ScopedClock({None: vc}))
    self.nc.all_engine_barrier()
    assert self.sems is not None
    popped = self.nc._tile_sem_poison_stack.pop()
    assert popped is self._sem_poison
    self.nc.clear_and_free_semaphores(list(self.sems.allocated().values()))
    self.nc.all_engine_barrier()


tile_mod.TileContext._drain_and_barrier = _split_drain_and_barrier


def _split_waits(nc, max_waits=1):
    for fn in nc.m.functions:
        for bb in fn.blocks:
            insts = bb.instructions
            if not any(
                i.sync_info and i.sync_info.on_wait
                and len(i.sync_info.on_wait) > max_waits
                for i in insts
            ):
                continue
            new = []
            for inst in insts:
                si = inst.sync_info
                ow = list(si.on_wait) if si and si.on_wait else []
                if len(ow) > max_waits:
                    excess, keep = ow[:-max_waits], ow[-max_waits:]
                    for j, w in enumerate(excess):
                        new.append(
                            mybir.InstEventSemaphore(
                                name=f"{inst.name}-wsplit{j}",
                                engine=inst.engine,
                                ins=[],
                                outs=[],
                                sync_info=mybir.SyncInfo(
                                    on_wait=[w], on_update=[]
                                ),
                            )
                        )
                    inst.sync_info = mybir.SyncInfo(
                        on_wait=keep, on_update=list(si.on_update or [])
                    )
                new.append(inst)
            bb.instructions = new


# ---------------------------------------------------------------------------
# Device program
# ---------------------------------------------------------------------------


def _build():
    NQ = S // 2
    n_qt = NQ // TQ          # 4
    n_kt_half = NQ // 128    # 16
    half = S // 2

    nc = bass.Bass()
    xH = nc.declare_dram_parameter("xH", [128, N_DIN * S], f16, isOutput=False)
    W = nc.declare_dram_parameter("W", [128, N_DIN * 3 * D], f16, isOutput=False)
    mask = nc.declare_dram_parameter("mask", [128, 256], f16, isOutput=False)
    out_num = nc.declare_dram_parameter("out_num", [D, NQ], f32, isOutput=True)
    out_den = nc.declare_dram_parameter("out_den", [128, 2 * NQ], f16, isOutput=True)

    with tile_mod.TileContext(nc) as tc:
        with (
            tc.tile_pool(name="persist", bufs=1) as persist,
            tc.tile_pool(name="work", bufs=8) as work,
            tc.tile_pool(name="sacc_p", bufs=2) as sacc_p,
            tc.tile_pool(name="outp", bufs=2) as outp,
            tc.tile_pool(name="ps_s", bufs=2, space="PSUM") as ps_s,
            tc.tile_pool(name="ps_o", bufs=2, space="PSUM") as ps_o,
            tc.tile_pool(name="ps_p", bufs=2, space="PSUM") as ps_p,
        ):
            x_all = persist.tile([128, N_DIN * S], f16, tag="x_all")
            # x is staged host-side in DMA-arrival block order: global 512-col
            # block g of the permuted x lives at position _POS[g], each block
            # holding all 6 d_in chunks contiguously -> every DMA phase is one
            # fully-contiguous range (128 descriptors of 6-12KB).
            _POS = {0: 0, 4: 1, 1: 2, 5: 3, 2: 4, 3: 5, 6: 6, 7: 7}

            def xs(di, g, lo=0, width=512):
                base = 3072 * _POS[g] + 512 * di + lo
                return x_all[:, base:base + width]
            w_all = persist.tile([128, N_DIN * 3 * D], f16, tag="w_all")
            m_all = persist.tile([128, 256], f16, tag="m_all")
            kt_sb = [persist.tile([128, 512], f16, tag=f"kt{c}", name=f"kt{c}")
                     for c in range(S // 512)]
            qt_sb = [persist.tile([128, TQ], f16, tag=f"qt{t}", name=f"qt{t}")
                     for t in range(n_qt)]
            v_sb = [persist.tile([128, D], f16, tag=f"v{k}", name=f"v{k}")
                    for k in range(2 * n_kt_half)]

            w_sb = [w_all[:, 3 * D * di:3 * D * (di + 1)] for di in range(N_DIN)]

            # --- input DMAs -------------------------------------------------
            # The GpSimd SWDGE queue sustains ~300GB/s (vs ~60GB/s for the
            # Sync/Scalar HWDGE queues), so ALL bulk input goes there, ordered
            # by first use: W, then x in contiguous arrival-order phases.
            nc.gpsimd.dma_start(out=w_all[:], in_=W[:])
            for lo, hi in ((0, 3072), (3072, 6144), (6144, 12288),
                           (12288, N_DIN * S)):
                nc.gpsimd.dma_start(out=x_all[:, lo:hi], in_=xH[:, lo:hi])
            nc.sync.dma_start(out=m_all[:], in_=mask[:])

            # PE pre-warm during the input-DMA wait: HAM un-throttles after
            # ~3.4us of sustained activity, so the first real matmuls run at
            # 2.4GHz instead of 1.2GHz
            warm_sb = persist.tile([128, 512], f16, tag="warm")
            nc.vector.memset(warm_sb[:], 0.0)
            psw = ps_p.tile([128, 512], f32, tag="p", name="warm_ps")
            for _ in range(22):
                nc.tensor.matmul(
                    psw[:], lhsT=warm_sb[:, 0:128], rhs=warm_sb[:],
                    start=True, stop=True,
                )

            def project_kt(c):
                ps = ps_p.tile([128, 512], f32, tag="p", name=f"pkt{c}")
                for di in range(N_DIN):
                    nc.tensor.matmul(
                        ps[:],
                        lhsT=w_sb[di][:, D:2 * D],
                        rhs=xs(di, c),
                        start=(di == 0),
                        stop=(di == N_DIN - 1),
                    )
                nc.vector.tensor_copy(kt_sb[c][:], ps[:])

            def project_qt(t):
                ps = ps_p.tile([128, 512], f32, tag="p", name=f"pqt{t}")
                for di in range(N_DIN):
                    nc.tensor.matmul(
                        ps[:],
                        lhsT=w_sb[di][:, 0:D],
                        rhs=xs(di, t),
                        start=(di == 0),
                        stop=(di == N_DIN - 1),
                    )
                nc.scalar.activation(qt_sb[t][:], ps[:], AF.Copy)

            def project_v_chunk(c):
                for k in range(4 * c, 4 * c + 4):
                    ps = ps_p.tile([128, D], f32, tag="p", name=f"pv{k}")
                    for di in range(N_DIN):
                        nc.tensor.matmul(
                            ps[:],
                            lhsT=xs(di, c, 128 * (k - 4 * c), 128),
                            rhs=w_sb[di][:, 2 * D:3 * D],
                            start=(di == 0),
                            stop=(di == N_DIN - 1),
                        )
                    nc.vector.tensor_copy(v_sb[k][:], ps[:])

            def emit_pair(t, i, kp, po, sacc, n_av):
                ps = ps_s.tile([128, 2 * TQ], f32, tag="s", name=f"s{t}_{kp}")
                pt = work.tile([128, 2 * TQ], f16, tag="pt", name=f"p{t}_{kp}")
                half2 = kp >= n_kt_half
                rel = kp - n_kt_half if half2 else kp
                diag = 4 * t <= rel < 4 * t + 4
                los = []
                for s_ in (0, 1):
                    kt = kp + s_
                    lo = 128 * (rel + s_ - 4 * t) if diag else 0
                    los.append(lo)
                    nc.tensor.matmul(
                        ps[:, TQ * s_ + lo:TQ * (s_ + 1)],
                        lhsT=kt_sb[kt // 4][:, 128 * (kt % 4):128 * (kt % 4 + 1)],
                        rhs=qt_sb[t][:, lo:TQ],
                        start=True,
                        stop=True,
                    )
                if diag:
                    wm = m_all[:, 128:256] if half2 else m_all[:, 0:128]
                    for s_ in (0, 1):
                        lo = los[s_]
                        nc.scalar.activation(
                            pt[:, TQ * s_ + lo:TQ * (s_ + 1)],
                            ps[:, TQ * s_ + lo:TQ * (s_ + 1)],
                            AF.Exp, scale=float(SCALE),
                        )
                        # zero the disallowed triangular wedge
                        nc.vector.tensor_mul(
                            pt[:, TQ * s_ + lo:TQ * s_ + lo + 128],
                            pt[:, TQ * s_ + lo:TQ * s_ + lo + 128],
                            wm,
                        )
                        nc.vector.tensor_add(
                            sacc[:, TQ * s_ + lo:TQ * (s_ + 1)],
                            sacc[:, TQ * s_ + lo:TQ * (s_ + 1)],
                            pt[:, TQ * s_ + lo:TQ * (s_ + 1)],
                        )
                else:
                    nc.scalar.activation(pt[:], ps[:], AF.Exp,
                                         scale=float(SCALE))
                    nc.vector.tensor_add(sacc[:], sacc[:], pt[:])
                for s_ in (0, 1):
                    kt = kp + s_
                    lo = los[s_]
                    nc.tensor.matmul(
                        po[:, lo:TQ],
                        lhsT=v_sb[kt][:],
                        rhs=pt[:, TQ * s_ + lo:TQ * (s_ + 1)],
                        start=(2 * i + s_ == 0),
                        stop=(2 * i + s_ == n_av - 1),
                    )

            for t in range(n_qt):
                po = ps_o.tile([128, TQ], f32, tag="o", name=f"po{t}")
                sacc = sacc_p.tile([128, 2 * TQ], f16, tag="sacc", name=f"sacc{t}")
                pairs = [2 * j for j in range(2 * (t + 1))] + [
                    n_kt_half + 2 * j for j in range(2 * (t + 1))
                ]
                n_av = 4 * (t + 1) * 2
                # Each q-tile needs only qt(t) projected up front; the
                # diagonal-chunk kt/v projections are emitted mid-phase, just
                # before the pairs that consume them. That gives the PE
                # independent work while ScalarE catches up on exps, and at
                # t=0 it matches the x-column DMA arrival order.
                project_qt(t)
                nc.vector.memset(sacc[:], 0.0)
                for i, kp in enumerate(pairs):
                    half2 = kp >= n_kt_half
                    rel = kp - n_kt_half if half2 else kp
                    if rel == 4 * t:  # diag chunk needed from here on
                        project_kt(t if not half2 else n_qt + t)
                        project_v_chunk(t if not half2 else n_qt + t)
                    emit_pair(t, i, kp, po, sacc, n_av)
                nc.gpsimd.dma_start(
                    out=out_den[:, 2 * TQ * t:2 * TQ * (t + 1)], in_=sacc[:]
                )
                ob = outp.tile([128, TQ], f32, tag="ob", name=f"ob{t}")
                nc.scalar.activation(ob[:], po[:], AF.Copy)
                nc.gpsimd.dma_start(out=out_num[:, TQ * t:TQ * (t + 1)], in_=ob[:])
    _split_waits(nc)
    return nc


_NC_CACHE = []


def _get_nc():
    if not _NC_CACHE:
        _NC_CACHE.append(_build())
    return _NC_CACHE[0]


def _host_inputs(x, Wq, Wk, Wv):
    W3 = np.concatenate([Wq, Wk, Wv], axis=1).astype(np.float16)  # [768, 384]
    W = np.ascontiguousarray(
        W3.reshape(N_DIN, 128, 3 * D).transpose(1, 0, 2).reshape(128, N_DIN * 3 * D)
    )
    u = np.arange(128)[:, None]
    c = np.arange(128)[None, :]
    masks = {}
    for h in (0, 1):
        w0 = (u <= c).astype(np.float16)          # first-half wedge
        w1 = (u <= c - 1 + h).astype(np.float16)  # second-half wedge
        masks[h] = np.ascontiguousarray(np.concatenate([w0, w1], axis=1))
    BLK = [0, 4, 1, 5, 2, 3, 6, 7]  # global 512-col block at each position
    in_maps = []
    for core in range(2 * B):
        b, h = divmod(core, 2)
        xp = np.concatenate([x[b, h::2], x[b, 1 - h::2]], axis=0)  # [S, 768]
        xT_p = xp.T.astype(np.float16)  # [768, S]
        x3 = xT_p.reshape(N_DIN, 128, 8, 512)          # [di, p, g, c]
        xh = x3.transpose(1, 2, 0, 3)[:, BLK]          # [p, pos, di, c]
        xh = np.ascontiguousarray(xh.reshape(128, N_DIN * S))
        in_maps.append({"xH": xh, "W": W, "mask": masks[h]})
    return in_maps


def kernel(x, Wq, Wk, Wv):
    x = np.asarray(x, np.float32)
    Wq = np.asarray(Wq, np.float32)
    Wk = np.asarray(Wk, np.float32)
    Wv = np.asarray(Wv, np.float32)
    nc = _get_nc()
    in_maps = _host_inputs(x, Wq, Wk, Wv)
    res = run_bass_kernel_spmd(nc, in_maps, list(range(2 * B)))
    out = np.empty((B, S, D), np.float32)
    NQ = S // 2
    for core in range(2 * B):
        b, h = divmod(core, 2)
        num = res.results[core]["out_num"]  # [128, NQ] f32
        sacc = res.results[core]["out_den"].astype(np.float32)  # [128, 2*NQ]
        s3 = sacc.reshape(128, NQ // TQ, 2, TQ)
        den = s3.sum(axis=(0, 2)).reshape(NQ)
        out[b, h::2, :] = (num / den[None, :]).T
    return out


# revision 16
# speedup vs baseline: 1.2532x; 1.0357x over previous
"""Causal-attention (QKV projection + softmax(QK^T/sqrt(d))V) on 8 trn2 cores.

Contract: kernel(x, Wq, Wk, Wv) takes FULL inputs
  x [4, 4096, 768] f32, Wq/Wk/Wv [768, 128] f32
and returns the FULL output [4, 4096, 128] f32.

Sharding: 2 cores per batch, split over KEY parity. Core with parity h of
batch b owns keys h::2 (2048 keys) and computes UNNORMALIZED partial
attention (numerator and exp-sum) for ALL 4096 queries against its keys;
the host adds the two cores' partials and divides. This halves the V
projection per core (K and Q projection volumes swap, a wash) with zero
cross-core communication, and key parity keeps the causal area balanced.

Per-core device program (fp16 matmuls, fp32 PSUM accumulation):
  K^T[d=128, 2048], V[k-tile][128 keys, 128 d], Q^T[d=128, 512] per q-tile;
  per 512-query tile: scores^T tiles [128 keys, 512 q] -> exp on ScalarE
  (no max subtraction: scores ~ N(0,1)). The causal boundary reduces to a
  [128,128] triangular wedge per diagonal key-tile, applied as a
  multiplicative 0/1 mask on VectorE after the exp; score matmuls / exp /
  sum accumulation are column-trimmed on diagonal tiles.
  Outputs: partial numerator OUT^T [128, 4096] f32 and exp-sum tiles
  [128, 8192] f16; the host reduces, combines core pairs, and divides.
"""
import numpy as np

import concourse.bass as bass
import concourse.mybir as mybir
import concourse.tile as tile_mod
from concourse.tile import ScopedClock, VectorClock
from concourse.tile_sem_assignment import N_PROCS
from concourse.bass_utils import run_bass_kernel_spmd

f32 = mybir.dt.float32
f16 = mybir.dt.float16

B, S, D_IN, D = 4, 4096, 768, 128
N_DIN = D_IN // 128  # 6
TQ = 512             # queries per q-tile
NK = S // 2          # keys per core
SCALE = 1.0 / np.sqrt(np.float32(D))
AF = mybir.ActivationFunctionType

# ---------------------------------------------------------------------------
# Workarounds: the walrus build in this container accepts only ONE sync-wait
# command per instruction. TileContext's exit drain carries one wait per
# active proc, and Tile's sem assignment emits multi-wait instructions.
# Split both onto single-wait carrier instructions.
# ---------------------------------------------------------------------------


def _split_drain_and_barrier(self, tick_clock, wait_clock):
    gc = tick_clock.global_clock
    for p in range(N_PROCS):
        if gc[p] == 0:
            continue
        vc = VectorClock([gc[q] if q == p else 0 for q in range(N_PROCS)])
        d = self.nc.sync.drain()
        wait_clock.add_sem_waits(d.ins, ScopedClock({None: vc}))
    self.nc.all_engine_barrier()
    assert self.sems is not None
    popped = self.nc._tile_sem_poison_stack.pop()
    assert popped is self._sem_poison
    self.nc.clear_and_free_semaphores(list(self.sems.allocated().values()))
    self.nc.all_engine_barrier()


tile_mod.TileContext._drain_and_barrier = _split_drain_and_barrier


def _split_waits(nc, max_waits=1):
    for fn in nc.m.functions:
        for bb in fn.blocks:
            insts = bb.instructions
            if not any(
                i.sync_info and i.sync_info.on_wait
                and len(i.sync_info.on_wait) > max_waits
                for i in insts
            ):
                continue
            new = []
            for inst in insts:
                si = inst.sync_info
                ow = list(si.on_wait) if si and si.on_wait else []
                if len(ow) > max_waits:
                    excess, keep = ow[:-max_waits], ow[-max_waits:]
                    for j, w in enumerate(excess):
                        new.append(
                            mybir.InstEventSemaphore(
                                name=f"{inst.name}-wsplit{j}",
                                engine=inst.engine,
                                ins=[],
                                outs=[],
                                sync_info=mybir.SyncInfo(
                                    on_wait=[w], on_update=[]
                                ),
                            )
                        )
                    inst.sync_info = mybir.SyncInfo(
                        on_wait=keep, on_update=list(si.on_update or [])
                    )
                new.append(inst)
            bb.instructions = new


# ---------------------------------------------------------------------------
# Device program
# ---------------------------------------------------------------------------

# q-tile processing order interleaves own-parity and other-parity tiles so
# the per-phase pair counts ramp 2,2,4,4,6,6,8,8 and x-block needs match the
# DMA arrival order below.
T_ORDER = [0, 4, 1, 5, 2, 6, 3, 7]
# global 512-col block g of the permuted x lives at host position _POS[g]
BLK = [0, 4, 1, 5, 2, 6, 3, 7]
_POS = {g: i for i, g in enumerate(BLK)}


def _build():
    n_kt = NK // 128  # 16 k-tiles of 128 keys

    nc = bass.Bass()
    xH = nc.declare_dram_parameter("xH", [128, N_DIN * S], f16, isOutput=False)
    W = nc.declare_dram_parameter("W", [128, N_DIN * 3 * D], f16, isOutput=False)
    mask = nc.declare_dram_parameter("mask", [128, 256], f16, isOutput=False)
    out_num = nc.declare_dram_parameter("out_num", [D, S], f32, isOutput=True)
    out_den = nc.declare_dram_parameter("out_den", [128, 2 * S], f16, isOutput=True)

    with tile_mod.TileContext(nc) as tc:
        with (
            tc.tile_pool(name="persist", bufs=1) as persist,
            tc.tile_pool(name="work", bufs=8) as work,
            tc.tile_pool(name="sacc_p", bufs=2) as sacc_p,
            tc.tile_pool(name="outp", bufs=2) as outp,
            tc.tile_pool(name="ps_s", bufs=2, space="PSUM") as ps_s,
            tc.tile_pool(name="ps_o", bufs=2, space="PSUM") as ps_o,
            tc.tile_pool(name="ps_p", bufs=2, space="PSUM") as ps_p,
        ):
            x_all = persist.tile([128, N_DIN * S], f16, tag="x_all")

            def xs(di, g, lo=0, width=512):
                base = 3072 * _POS[g] + 512 * di + lo
                return x_all[:, base:base + width]

            w_all = persist.tile([128, N_DIN * 3 * D], f16, tag="w_all")
            m_all = persist.tile([128, 256], f16, tag="m_all")
            kt_sb = [persist.tile([128, 512], f16, tag=f"kt{c}", name=f"kt{c}")
                     for c in range(NK // 512)]
            qt_sb = [persist.tile([128, TQ], f16, tag=f"qt{t}", name=f"qt{t}")
                     for t in range(8)]
            v_sb = [persist.tile([128, D], f16, tag=f"v{k}", name=f"v{k}")
                    for k in range(n_kt)]

            w_sb = [w_all[:, 3 * D * di:3 * D * (di + 1)] for di in range(N_DIN)]

            # --- input DMAs -------------------------------------------------
            # All bulk input on the GpSimd SWDGE queue (~300GB/s vs ~60GB/s
            # for Sync/Scalar HWDGE), ordered by first use, each phase one
            # fully-contiguous range thanks to the host-side x relayout.
            nc.gpsimd.dma_start(out=w_all[:], in_=W[:])
            for lo, hi in ((0, 3072), (3072, 6144), (6144, 12288),
                           (12288, N_DIN * S)):
                nc.gpsimd.dma_start(out=x_all[:, lo:hi], in_=xH[:, lo:hi])
            nc.sync.dma_start(out=m_all[:], in_=mask[:])

            # PE pre-warm during the input-DMA wait: HAM un-throttles after
            # ~3.4us of sustained activity, so the first real matmuls run at
            # 2.4GHz instead of 1.2GHz
            warm_sb = persist.tile([128, 512], f16, tag="warm")
            nc.vector.memset(warm_sb[:], 0.0)
            psw = ps_p.tile([128, 512], f32, tag="p", name="warm_ps")
            for _ in range(16):
                nc.tensor.matmul(
                    psw[:], lhsT=warm_sb[:, 0:128], rhs=warm_sb[:],
                    start=True, stop=True,
                )

            def project_kt(c):
                ps = ps_p.tile([128, 512], f32, tag="p", name=f"pkt{c}")
                for di in range(N_DIN):
                    nc.tensor.matmul(
                        ps[:],
                        lhsT=w_sb[di][:, D:2 * D],
                        rhs=xs(di, c),
                        start=(di == 0),
                        stop=(di == N_DIN - 1),
                    )
                nc.vector.tensor_copy(kt_sb[c][:], ps[:])

            def project_qt(t):
                ps = ps_p.tile([128, 512], f32, tag="p", name=f"pqt{t}")
                for di in range(N_DIN):
                    nc.tensor.matmul(
                        ps[:],
                        lhsT=w_sb[di][:, 0:D],
                        rhs=xs(di, t),
                        start=(di == 0),
                        stop=(di == N_DIN - 1),
                    )
                nc.scalar.activation(qt_sb[t][:], ps[:], AF.Copy)

            def project_v_chunk(c):
                for k in range(4 * c, 4 * c + 4):
                    ps = ps_p.tile([128, D], f32, tag="p", name=f"pv{k}")
                    for di in range(N_DIN):
                        nc.tensor.matmul(
                            ps[:],
                            lhsT=xs(di, c, 128 * (k - 4 * c), 128),
                            rhs=w_sb[di][:, 2 * D:3 * D],
                            start=(di == 0),
                            stop=(di == N_DIN - 1),
                        )
                    nc.vector.tensor_copy(v_sb[k][:], ps[:])

            def emit_pair(t, tt, i, kp, po, sacc, n_av, wm):
                ps = ps_s.tile([128, 2 * TQ], f32, tag="s", name=f"s{t}_{kp}")
                pt = work.tile([128, 2 * TQ], f16, tag="pt", name=f"p{t}_{kp}")
                diag = 4 * tt <= kp < 4 * tt + 4
                los = []
                for s_ in (0, 1):
                    kt = kp + s_
                    lo = 128 * (kp + s_ - 4 * tt) if diag else 0
                    los.append(lo)
                    nc.tensor.matmul(
                        ps[:, TQ * s_ + lo:TQ * (s_ + 1)],
                        lhsT=kt_sb[kt // 4][:, 128 * (kt % 4):128 * (kt % 4 + 1)],
                        rhs=qt_sb[t][:, lo:TQ],
                        start=True,
                        stop=True,
                    )
                if diag:
                    for s_ in (0, 1):
                        lo = los[s_]
                        nc.scalar.activation(
                            pt[:, TQ * s_ + lo:TQ * (s_ + 1)],
                            ps[:, TQ * s_ + lo:TQ * (s_ + 1)],
                            AF.Exp, scale=float(SCALE),
                        )
                        # zero the disallowed triangular wedge
                        nc.vector.tensor_mul(
                            pt[:, TQ * s_ + lo:TQ * s_ + lo + 128],
                            pt[:, TQ * s_ + lo:TQ * s_ + lo + 128],
                            wm,
                        )
                        nc.vector.tensor_add(
                            sacc[:, TQ * s_ + lo:TQ * (s_ + 1)],
                            sacc[:, TQ * s_ + lo:TQ * (s_ + 1)],
                            pt[:, TQ * s_ + lo:TQ * (s_ + 1)],
                        )
                else:
                    nc.scalar.activation(pt[:], ps[:], AF.Exp,
                                         scale=float(SCALE))
                    nc.vector.tensor_add(sacc[:], sacc[:], pt[:])
                for s_ in (0, 1):
                    kt = kp + s_
                    lo = los[s_]
                    nc.tensor.matmul(
                        po[:, lo:TQ],
                        lhsT=v_sb[kt][:],
                        rhs=pt[:, TQ * s_ + lo:TQ * (s_ + 1)],
                        start=(2 * i + s_ == 0),
                        stop=(2 * i + s_ == n_av - 1),
                    )

            for t in T_ORDER:
                tt = t if t < 4 else t - 4
                own = t < 4
                wm = m_all[:, 0:128] if own else m_all[:, 128:256]
                po = ps_o.tile([128, TQ], f32, tag="o", name=f"po{t}")
                sacc = sacc_p.tile([128, 2 * TQ], f16, tag="sacc",
                                   name=f"sacc{t}")
                pairs = [2 * j for j in range(2 * (tt + 1))]
                n_av = len(pairs) * 2
                project_qt(t)
                nc.vector.memset(sacc[:], 0.0)
                for i, kp in enumerate(pairs):
                    if own and kp == 4 * tt:
                        # diag chunk projections, emitted just before the
                        # pairs that consume them (keeps independent PE work
                        # available while ScalarE catches up on exps)
                        project_kt(tt)
                        project_v_chunk(tt)
                    emit_pair(t, tt, i, kp, po, sacc, n_av, wm)
                nc.gpsimd.dma_start(
                    out=out_den[:, 2 * TQ * t:2 * TQ * (t + 1)], in_=sacc[:]
                )
                ob = outp.tile([128, TQ], f32, tag="ob", name=f"ob{t}")
                nc.scalar.activation(ob[:], po[:], AF.Copy)
                nc.gpsimd.dma_start(out=out_num[:, TQ * t:TQ * (t + 1)], in_=ob[:])
    _split_waits(nc)
    return nc


_NC_CACHE = []


def _get_nc():
    if not _NC_CACHE:
        _NC_CACHE.append(_build())
    return _NC_CACHE[0]


def _host_inputs(x, Wq, Wk, Wv):
    W3 = np.concatenate([Wq, Wk, Wv], axis=1).astype(np.float16)  # [768, 384]
    W = np.ascontiguousarray(
        W3.reshape(N_DIN, 128, 3 * D).transpose(1, 0, 2).reshape(128, N_DIN * 3 * D)
    )
    u = np.arange(128)[:, None]
    c = np.arange(128)[None, :]
    masks = {}
    for h in (0, 1):
        w_own = (u <= c).astype(np.float16)
        w_oth = (u <= c - h).astype(np.float16)
        masks[h] = np.ascontiguousarray(np.concatenate([w_own, w_oth], axis=1))
    in_maps = []
    for core in range(2 * B):
        b, h = divmod(core, 2)
        xp = np.concatenate([x[b, h::2], x[b, 1 - h::2]], axis=0)  # [S, 768]
        xT_p = xp.T.astype(np.float16)  # [768, S]
        x3 = xT_p.reshape(N_DIN, 128, 8, 512)          # [di, p, g, c]
        xh = x3.transpose(1, 2, 0, 3)[:, BLK]          # [p, pos, di, c]
        xh = np.ascontiguousarray(xh.reshape(128, N_DIN * S))
        in_maps.append({"xH": xh, "W": W, "mask": masks[h]})
    return in_maps


def kernel(x, Wq, Wk, Wv):
    x = np.asarray(x, np.float32)
    Wq = np.asarray(Wq, np.float32)
    Wk = np.asarray(Wk, np.float32)
    Wv = np.asarray(Wv, np.float32)
    nc = _get_nc()
    in_maps = _host_inputs(x, Wq, Wk, Wv)
    res = run_bass_kernel_spmd(nc, in_maps, list(range(2 * B)))
    out = np.empty((B, S, D), np.float32)
    for b in range(B):
        num = {}
        den = {}
        for h in (0, 1):
            r = res.results[2 * b + h]
            n = r["out_num"]                                  # [128, S] f32
            sacc = r["out_den"].astype(np.float32)            # [128, 2S]
            dd = sacc.reshape(128, 8, 2, TQ).sum(axis=(0, 2)).reshape(S)
            # query index qi of core h -> original row:
            #   qi < 2048: row 2*qi + h ; qi >= 2048: row 2*(qi-2048) + 1-h
            na = np.empty((128, S), np.float32)
            da = np.empty(S, np.float32)
            na[:, h::2] = n[:, :S // 2]
            na[:, 1 - h::2] = n[:, S // 2:]
            da[h::2] = dd[:S // 2]
            da[1 - h::2] = dd[S // 2:]
            num[h] = na
            den[h] = da
        out[b] = ((num[0] + num[1]) / (den[0] + den[1])[None, :]).T
    return out


# revision 23
# speedup vs baseline: 1.2754x; 1.0177x over previous
"""Causal-attention (QKV projection + softmax(QK^T/sqrt(d))V) on 8 trn2 cores.

Contract: kernel(x, Wq, Wk, Wv) takes FULL inputs
  x [4, 4096, 768] f32, Wq/Wk/Wv [768, 128] f32
and returns the FULL output [4, 4096, 128] f32.

Sharding: 2 cores per batch, split over KEY parity. Core with parity h of
batch b owns keys h::2 (2048 keys) and computes UNNORMALIZED partial
attention (numerator and exp-sum) for ALL 4096 queries against its keys;
the host adds the two cores' partials and divides. This halves the V
projection per core (K and Q projection volumes swap, a wash) with zero
cross-core communication, and key parity keeps the causal area balanced.

Per-core device program (fp16 matmuls, fp32 PSUM accumulation):
  K^T[d=128, 2048], V[k-tile][128 keys, 128 d], Q^T[d=128, 512] per q-tile;
  per 512-query tile: scores^T tiles [128 keys, 512 q] -> exp on ScalarE
  (no max subtraction: scores ~ N(0,1)). The causal boundary reduces to a
  [128,128] triangular wedge per diagonal key-tile, applied as a
  multiplicative 0/1 mask on VectorE after the exp; score matmuls / exp /
  sum accumulation are column-trimmed on diagonal tiles.
  Outputs: partial numerator OUT^T [128, 4096] f32 and exp-sum tiles
  [128, 8192] f16; the host reduces, combines core pairs, and divides.
"""
import numpy as np

import concourse.bass as bass
import concourse.mybir as mybir
import concourse.tile as tile_mod
from concourse.tile import ScopedClock, VectorClock
from concourse.tile_sem_assignment import N_PROCS
from concourse.bass_utils import run_bass_kernel_spmd

f32 = mybir.dt.float32
f16 = mybir.dt.float16

B, S, D_IN, D = 4, 4096, 768, 128
N_DIN = D_IN // 128  # 6
TQ = 512             # queries per q-tile
NK = S // 2          # keys per core
SCALE = 1.0 / np.sqrt(np.float32(D))
AF = mybir.ActivationFunctionType

# ---------------------------------------------------------------------------
# Workarounds: the walrus build in this container accepts only ONE sync-wait
# command per instruction. TileContext's exit drain carries one wait per
# active proc, and Tile's sem assignment emits multi-wait instructions.
# Split both onto single-wait carrier instructions.
# ---------------------------------------------------------------------------


def _split_drain_and_barrier(self, tick_clock, wait_clock):
    gc = tick_clock.global_clock
    for p in range(N_PROCS):
        if gc[p] == 0:
            continue
        vc = VectorClock([gc[q] if q == p else 0 for q in range(N_PROCS)])
        d = self.nc.sync.drain()
        wait_clock.add_sem_waits(d.ins, ScopedClock({None: vc}))
    self.nc.all_engine_barrier()
    assert self.sems is not None
    popped = self.nc._tile_sem_poison_stack.pop()
    assert popped is self._sem_poison
    self.nc.clear_and_free_semaphores(list(self.sems.allocated().values()))
    self.nc.all_engine_barrier()


tile_mod.TileContext._drain_and_barrier = _split_drain_and_barrier


def _split_waits(nc, max_waits=1):
    for fn in nc.m.functions:
        for bb in fn.blocks:
            insts = bb.instructions
            if not any(
                i.sync_info and i.sync_info.on_wait
                and len(i.sync_info.on_wait) > max_waits
                for i in insts
            ):
                continue
            new = []
            for inst in insts:
                si = inst.sync_info
                ow = list(si.on_wait) if si and si.on_wait else []
                if len(ow) > max_waits:
                    excess, keep = ow[:-max_waits], ow[-max_waits:]
                    for j, w in enumerate(excess):
                        new.append(
                            mybir.InstEventSemaphore(
                                name=f"{inst.name}-wsplit{j}",
                                engine=inst.engine,
                                ins=[],
                                outs=[],
                                sync_info=mybir.SyncInfo(
                                    on_wait=[w], on_update=[]
                                ),
                            )
                        )
                    inst.sync_info = mybir.SyncInfo(
                        on_wait=keep, on_update=list(si.on_update or [])
                    )
                new.append(inst)
            bb.instructions = new


# ---------------------------------------------------------------------------
# Device program
# ---------------------------------------------------------------------------

# q-tile processing order interleaves own-parity and other-parity tiles so
# the per-phase pair counts ramp 2,2,4,4,6,6,8,8 and x-block needs match the
# DMA arrival order below.
T_ORDER = [0, 4, 1, 5, 2, 6, 3, 7]
# global 512-col block g of the permuted x lives at host position _POS[g]
BLK = [0, 4, 1, 5, 2, 6, 3, 7]
_POS = {g: i for i, g in enumerate(BLK)}


def _build():
    n_kt = NK // 128  # 16 k-tiles of 128 keys

    nc = bass.Bass()
    xH = nc.declare_dram_parameter("xH", [128, N_DIN * S], f16, isOutput=False)
    W = nc.declare_dram_parameter("W", [128, N_DIN * 3 * D], f16, isOutput=False)
    mask = nc.declare_dram_parameter("mask", [128, 256], f16, isOutput=False)
    out_num = nc.declare_dram_parameter("out_num", [D, S], f32, isOutput=True)
    out_den = nc.declare_dram_parameter("out_den", [128, 2 * S], f16, isOutput=True)

    with tile_mod.TileContext(nc) as tc:
        with (
            tc.tile_pool(name="persist", bufs=1) as persist,
            tc.tile_pool(name="work", bufs=8) as work,
            tc.tile_pool(name="sacc_p", bufs=2) as sacc_p,
            tc.tile_pool(name="outp", bufs=2) as outp,
            tc.tile_pool(name="ps_s", bufs=2, space="PSUM") as ps_s,
            tc.tile_pool(name="ps_o", bufs=2, space="PSUM") as ps_o,
            tc.tile_pool(name="ps_p", bufs=2, space="PSUM") as ps_p,
        ):
            x_all = persist.tile([128, N_DIN * S], f16, tag="x_all")

            def xs(di, g, lo=0, width=512):
                base = 3072 * _POS[g] + 512 * di + lo
                return x_all[:, base:base + width]

            w_all = persist.tile([128, N_DIN * 3 * D], f16, tag="w_all")
            m_all = persist.tile([128, 256], f16, tag="m_all")
            kt_sb = [persist.tile([128, 512], f16, tag=f"kt{c}", name=f"kt{c}")
                     for c in range(NK // 512)]
            qt_sb = [persist.tile([128, TQ], f16, tag=f"qt{t}", name=f"qt{t}")
                     for t in range(8)]
            v_sb = [persist.tile([128, D], f16, tag=f"v{k}", name=f"v{k}")
                    for k in range(n_kt)]

            w_sb = [w_all[:, 3 * D * di:3 * D * (di + 1)] for di in range(N_DIN)]

            # --- input DMAs -------------------------------------------------
            # All bulk input on the GpSimd SWDGE queue (~300GB/s vs ~60GB/s
            # for Sync/Scalar HWDGE), ordered by first use, each phase one
            # fully-contiguous range thanks to the host-side x relayout.
            nc.gpsimd.dma_start(out=w_all[:], in_=W[:])
            for lo, hi in ((0, 3072), (3072, 6144), (6144, 12288),
                           (12288, N_DIN * S)):
                nc.gpsimd.dma_start(out=x_all[:, lo:hi], in_=xH[:, lo:hi])
            nc.sync.dma_start(out=m_all[:], in_=mask[:])

            # PE pre-warm during the input-DMA wait: HAM un-throttles after
            # ~3.4us of sustained activity, so the first real matmuls run at
            # 2.4GHz instead of 1.2GHz
            warm_sb = persist.tile([128, 512], f16, tag="warm")
            nc.vector.memset(warm_sb[:], 0.0)
            psw = ps_p.tile([128, 512], f32, tag="p", name="warm_ps")
            for _ in range(16):
                nc.tensor.matmul(
                    psw[:], lhsT=warm_sb[:, 0:128], rhs=warm_sb[:],
                    start=True, stop=True,
                )

            def project_kt(c):
                ps = ps_p.tile([128, 512], f32, tag="p", name=f"pkt{c}")
                for di in range(N_DIN):
                    nc.tensor.matmul(
                        ps[:],
                        lhsT=w_sb[di][:, D:2 * D],
                        rhs=xs(di, c),
                        start=(di == 0),
                        stop=(di == N_DIN - 1),
                    )
                nc.vector.tensor_copy(kt_sb[c][:], ps[:])

            def project_qt(t):
                ps = ps_p.tile([128, 512], f32, tag="p", name=f"pqt{t}")
                for di in range(N_DIN):
                    nc.tensor.matmul(
                        ps[:],
                        lhsT=w_sb[di][:, 0:D],
                        rhs=xs(di, t),
                        start=(di == 0),
                        stop=(di == N_DIN - 1),
                    )
                nc.scalar.activation(qt_sb[t][:], ps[:], AF.Copy)

            def project_v_chunk(c):
                for k in range(4 * c, 4 * c + 4):
                    ps = ps_p.tile([128, D], f32, tag="p", name=f"pv{k}")
                    for di in range(N_DIN):
                        nc.tensor.matmul(
                            ps[:],
                            lhsT=xs(di, c, 128 * (k - 4 * c), 128),
                            rhs=w_sb[di][:, 2 * D:3 * D],
                            start=(di == 0),
                            stop=(di == N_DIN - 1),
                        )
                    nc.vector.tensor_copy(v_sb[k][:], ps[:])

            def emit_pair(t, tt, i, kp, po, sacc, n_av, wm):
                ps = ps_s.tile([128, 2 * TQ], f32, tag="s", name=f"s{t}_{kp}")
                pt = work.tile([128, 2 * TQ], f16, tag="pt", name=f"p{t}_{kp}")
                diag = 4 * tt <= kp < 4 * tt + 4
                los = []
                for s_ in (0, 1):
                    kt = kp + s_
                    lo = 128 * (kp + s_ - 4 * tt) if diag else 0
                    los.append(lo)
                    nc.tensor.matmul(
                        ps[:, TQ * s_ + lo:TQ * (s_ + 1)],
                        lhsT=kt_sb[kt // 4][:, 128 * (kt % 4):128 * (kt % 4 + 1)],
                        rhs=qt_sb[t][:, lo:TQ],
                        start=True,
                        stop=True,
                    )
                first = i == 0
                if diag:
                    for s_ in (0, 1):
                        lo = los[s_]
                        nc.scalar.activation(
                            pt[:, TQ * s_ + lo:TQ * (s_ + 1)],
                            ps[:, TQ * s_ + lo:TQ * (s_ + 1)],
                            AF.Exp, scale=float(SCALE),
                        )
                        # zero the disallowed triangular wedge
                        nc.vector.tensor_mul(
                            pt[:, TQ * s_ + lo:TQ * s_ + lo + 128],
                            pt[:, TQ * s_ + lo:TQ * s_ + lo + 128],
                            wm,
                        )
                        if first:
                            # first pair initializes sacc: copy the live
                            # range, zero the trimmed prefix
                            if lo:
                                nc.vector.memset(
                                    sacc[:, TQ * s_:TQ * s_ + lo], 0.0
                                )
                            nc.vector.tensor_copy(
                                sacc[:, TQ * s_ + lo:TQ * (s_ + 1)],
                                pt[:, TQ * s_ + lo:TQ * (s_ + 1)],
                            )
                        else:
                            nc.vector.tensor_add(
                                sacc[:, TQ * s_ + lo:TQ * (s_ + 1)],
                                sacc[:, TQ * s_ + lo:TQ * (s_ + 1)],
                                pt[:, TQ * s_ + lo:TQ * (s_ + 1)],
                            )
                else:
                    nc.scalar.activation(pt[:], ps[:], AF.Exp,
                                         scale=float(SCALE))
                    if first:
                        nc.vector.tensor_copy(sacc[:], pt[:])
                    else:
                        nc.vector.tensor_add(sacc[:], sacc[:], pt[:])
                for s_ in (0, 1):
                    kt = kp + s_
                    lo = los[s_]
                    nc.tensor.matmul(
                        po[:, lo:TQ],
                        lhsT=v_sb[kt][:],
                        rhs=pt[:, TQ * s_ + lo:TQ * (s_ + 1)],
                        start=(2 * i + s_ == 0),
                        stop=(2 * i + s_ == n_av - 1),
                    )

            for t in T_ORDER:
                tt = t if t < 4 else t - 4
                own = t < 4
                wm = m_all[:, 0:128] if own else m_all[:, 128:256]
                po = ps_o.tile([128, TQ], f32, tag="o", name=f"po{t}")
                sacc = sacc_p.tile([128, 2 * TQ], f16, tag="sacc",
                                   name=f"sacc{t}")
                pairs = [2 * j for j in range(2 * (tt + 1))]
                n_av = len(pairs) * 2
                project_qt(t)
                for i, kp in enumerate(pairs):
                    if own and kp == 4 * tt:
                        # diag chunk projections, emitted just before the
                        # pairs that consume them (keeps independent PE work
                        # available while ScalarE catches up on exps)
                        project_kt(tt)
                        project_v_chunk(tt)
                    emit_pair(t, tt, i, kp, po, sacc, n_av, wm)
                nc.gpsimd.dma_start(
                    out=out_den[:, 2 * TQ * t:2 * TQ * (t + 1)], in_=sacc[:]
                )
                ob = outp.tile([128, TQ], f32, tag="ob", name=f"ob{t}")
                nc.scalar.activation(ob[:], po[:], AF.Copy)
                nc.gpsimd.dma_start(out=out_num[:, TQ * t:TQ * (t + 1)], in_=ob[:])
    _split_waits(nc)
    return nc


_NC_CACHE = []


def _get_nc():
    if not _NC_CACHE:
        _NC_CACHE.append(_build())
    return _NC_CACHE[0]


def _host_inputs(x, Wq, Wk, Wv):
    W3 = np.concatenate([Wq, Wk, Wv], axis=1).astype(np.float16)  # [768, 384]
    W = np.ascontiguousarray(
        W3.reshape(N_DIN, 128, 3 * D).transpose(1, 0, 2).reshape(128, N_DIN * 3 * D)
    )
    u = np.arange(128)[:, None]
    c = np.arange(128)[None, :]
    masks = {}
    for h in (0, 1):
        w_own = (u <= c).astype(np.float16)
        w_oth = (u <= c - h).astype(np.float16)
        masks[h] = np.ascontiguousarray(np.concatenate([w_own, w_oth], axis=1))
    in_maps = []
    for core in range(2 * B):
        b, h = divmod(core, 2)
        xp = np.concatenate([x[b, h::2], x[b, 1 - h::2]], axis=0)  # [S, 768]
        xT_p = xp.T.astype(np.float16)  # [768, S]
        x3 = xT_p.reshape(N_DIN, 128, 8, 512)          # [di, p, g, c]
        xh = x3.transpose(1, 2, 0, 3)[:, BLK]          # [p, pos, di, c]
        xh = np.ascontiguousarray(xh.reshape(128, N_DIN * S))
        in_maps.append({"xH": xh, "W": W, "mask": masks[h]})
    return in_maps


def kernel(x, Wq, Wk, Wv):
    x = np.asarray(x, np.float32)
    Wq = np.asarray(Wq, np.float32)
    Wk = np.asarray(Wk, np.float32)
    Wv = np.asarray(Wv, np.float32)
    nc = _get_nc()
    in_maps = _host_inputs(x, Wq, Wk, Wv)
    res = run_bass_kernel_spmd(nc, in_maps, list(range(2 * B)))
    out = np.empty((B, S, D), np.float32)
    for b in range(B):
        num = {}
        den = {}
        for h in (0, 1):
            r = res.results[2 * b + h]
            n = r["out_num"]                                  # [128, S] f32
            sacc = r["out_den"].astype(np.float32)            # [128, 2S]
            dd = sacc.reshape(128, 8, 2, TQ).sum(axis=(0, 2)).reshape(S)
            # query index qi of core h -> original row:
            #   qi < 2048: row 2*qi + h ; qi >= 2048: row 2*(qi-2048) + 1-h
            na = np.empty((128, S), np.float32)
            da = np.empty(S, np.float32)
            na[:, h::2] = n[:, :S // 2]
            na[:, 1 - h::2] = n[:, S // 2:]
            da[h::2] = dd[:S // 2]
            da[1 - h::2] = dd[S // 2:]
            num[h] = na
            den[h] = da
        out[b] = ((num[0] + num[1]) / (den[0] + den[1])[None, :]).T
    return out
